# revision 5
# baseline (speedup 1.0000x reference)
"""Trainium2 Bass kernel for nn_DiffForest (soft decision forest forward).

Math: per tree t, z = x @ w_d[t]; p = sigmoid(z); leaf path probs are products
of 8 factors p/(1-p) down a depth-8 tree; output = sum_t leaf_prob @ softmax(w_l[t]) / 10.

Kernel formulation (all on device except small weight prep):
  - The 512 "leaves" come in identical pairs -> fold to 256 paths; fold the
    pair-sum + 1/n_trees into the leaf weight matrix w2 (host, exact).
  - Path products move to log space:  -log P[q] = sum_path softplus(-z) + sum_{branch=1} z
    which is a matmul with a constant matrix S [512, 256]:
        A = S^T @ [softplus(-z); z],   leaf_prob^T = exp(-A)   ([256 paths, batch])
    softplus(-z) = ln(1 + exp(-z)) via the Exp/Ln activation tables.
  - Decision matmul: mixed precision. 12 of 16 k-tiles run fp8e4 (x scaled by
    16, w_d by 64) with MatmulPerfMode.DoubleRow (two k-tiles per pass, 2x PE
    throughput); the last 4 k-tiles run f16 at the same 1024 product scale and
    accumulate into the same PSUM. Full-fp8 measures 1.92e-2 rel err vs the
    2e-2 gate; the f16 fraction buys the error margin back (sqrt(12/16) x).
    The 1/1024 descale folds into the Exp activation scale and the z-half of S.
  - S-matmul in fp32r; leaf matmul in f16 (fp8 there costs ~1.2e-2 rel err).
  - All dram tensors are host-pre-tiled so every DMA moves long contiguous
    lines per partition (256B lines measured ~200GB/s; 4-8KB lines fix that).
  - The S-matmul block of group g is emitted after two trees of group g+1 so
    the in-order PE rides through the group's Exp->Ln->table-load latency.
  - Sharding: data-parallel over batch; each of the 8 cores takes 2048 rows of x,
    weights replicated, no collectives.
"""

import numpy as np
import ml_dtypes

import concourse.bacc as bacc
import concourse.mybir as mybir
import concourse.tile as tile
from concourse.tile import add_dep_helper
from concourse.bass_utils import run_bass_kernel_spmd

N_CORES = 8
BATCH = 16384
B_LOC = BATCH // N_CORES        # 2048 rows per core
IN_DIM = 2048
N_TREES = 10
ND_PAD = 256                    # decision nodes padded 255 -> 256
NQ = 256                        # folded path (leaf) count
CLASSES = 1000
CHUNK = 512                     # batch columns processed per chunk
KI = IN_DIM // 128              # 16 contraction tiles for the decision matmul
K8 = 12                         # k-tiles in fp8 (DoubleRow pairs)
K16 = KI - K8                   # k-tiles in f16
N_CHUNKS = B_LOC // CHUNK

BF16 = mybir.dt.bfloat16
F32 = mybir.dt.float32
F32R = mybir.dt.float32r
F16 = mybir.dt.float16
F8 = mybir.dt.float8e4
AF = mybir.ActivationFunctionType
DR = mybir.MatmulPerfMode.DoubleRow

X_SCALE = 16.0                  # x -> fp8/f16
WD_SCALE = 64.0                 # w_d -> fp8/f16
Z_DESCALE = 1.0 / (X_SCALE * WD_SCALE)   # psum holds 1024*z

_CACHE = {}


def _build(b_loc=B_LOC, n_trees=N_TREES):
    n_chunks = b_loc // CHUNK
    nc = bacc.Bacc("TRN2", target_bir_lowering=False)
    # host-pre-tiled layouts: partition dim first, contiguous k*free lines
    xt8 = nc.dram_tensor("xt8", (128, n_chunks, K8, CHUNK), F8, kind="ExternalInput")
    xt16 = nc.dram_tensor(
        "xt16", (128, n_chunks, K16, CHUNK), F16, kind="ExternalInput"
    )
    wd8 = nc.dram_tensor("wd8", (n_trees, 128, K8, ND_PAD), F8, kind="ExternalInput")
    wd16 = nc.dram_tensor(
        "wd16", (n_trees, 128, K16, ND_PAD), F16, kind="ExternalInput"
    )
    smat = nc.dram_tensor("smat", (128, 4, NQ), F32R, kind="ExternalInput")
    w2 = nc.dram_tensor("w2", (n_trees, 128, 2, CLASSES), F16, kind="ExternalInput")
    out = nc.dram_tensor("out", (b_loc, CLASSES), F32, kind="ExternalOutput")

    with tile.TileContext(nc) as tc:
        with (
            tc.tile_pool(name="const", bufs=1) as constp,
            tc.tile_pool(name="sb", bufs=2) as sb,
            tc.tile_pool(name="ep", bufs=5) as ep,
            tc.tile_pool(name="gp", bufs=5) as gp,
            tc.tile_pool(name="outp", bufs=2) as outp,
            tc.tile_pool(name="lptp", bufs=1) as lptp,
            tc.tile_pool(name="pz", bufs=2, space="PSUM") as pzp,
            tc.tile_pool(name="plp", bufs=2, space="PSUM") as plpp,
            tc.tile_pool(name="po", bufs=2, space="PSUM") as pop,
        ):
            smat_sb = constp.tile([128, 4, NQ], F32R)
            w2_sb = constp.tile([128, n_trees, 2, CLASSES], F16)

            GROUP = 5
            first_mm = [None]

            # chunk-0 x load first: it heads the critical path
            xt_tiles = [None] * n_chunks

            def load_x(ci):
                xa = sb.tile([128, K8, CHUNK], F8, tag="xt8")
                nc.sync.dma_start(xa[:, :, :], xt8[:, ci, :, :])
                xb = sb.tile([128, K16, CHUNK], F16, tag="xt16")
                nc.sync.dma_start(xb[:, :, :], xt16[:, ci, :, :])
                xt_tiles[ci] = (xa, xb)

            load_x(0)
            # tree-0 weights second
            wd_tiles = [None] * n_trees

            def load_wd(t):
                wa = constp.tile([128, K8, ND_PAD], F8, tag=f"wd8_{t}")
                d1 = nc.sync.dma_start(wa[:, :, :], wd8[t, :, :, :])
                wb = constp.tile([128, K16, ND_PAD], F16, tag=f"wd16_{t}")
                d2 = nc.sync.dma_start(wb[:, :, :], wd16[t, :, :, :])
                wd_tiles[t] = (wa, wb)
                return d1, d2

            load_wd(0)

            def emit_deferred():
                # behind the first matmul so they can't crowd the startup queues
                dmas = [nc.sync.dma_start(smat_sb[:, :, :], smat[:, :, :])]
                for t in range(1, n_trees):
                    dmas.extend(load_wd(t))
                for t in range(n_trees):
                    dmas.append(
                        nc.sync.dma_start(w2_sb[:, t, :, :], w2[t, :, :, :])
                    )
                for dma in dmas:
                    add_dep_helper(
                        dma.ins, first_mm[0].ins, sync=True,
                        reason="startup: critical pieces first",
                    )

            def emit_decision(t, xp, lpT):
                """mixed fp8-DoubleRow / f16 decision matmuls + Exp/z-copy."""
                xa, xb = xp
                wa, wb = wd_tiles[t]
                G = gp.tile([128, 4, CHUNK], F32R, tag="G")
                E = ep.tile([128, 2, CHUNK], F16, tag="E")
                last_exp = None
                fresh = first_mm[0] is None
                for dt_ in range(2):
                    psz = pzp.tile([128, CHUNK], F32, tag="psz")
                    for j in range(K8 // 2):
                        mm = nc.tensor.matmul(
                            psz[:, :],
                            wa[:, 2 * j : 2 * j + 2, dt_ * 128 : (dt_ + 1) * 128],
                            xa[:, 2 * j : 2 * j + 2, :],
                            start=(j == 0),
                            stop=False,
                            perf_mode=DR,
                        )
                        if first_mm[0] is None:
                            first_mm[0] = mm
                    for j in range(K16):
                        nc.tensor.matmul(
                            psz[:, :],
                            wb[:, j, dt_ * 128 : (dt_ + 1) * 128],
                            xb[:, j, :],
                            start=False,
                            stop=(j == K16 - 1),
                        )
                    last_exp = nc.scalar.activation(
                        E[:, dt_, :], psz[:, :], AF.Exp, scale=-Z_DESCALE
                    )
                    nc.vector.tensor_copy(G[:, 2 + dt_, :], psz[:, :])
                if fresh:
                    emit_deferred()
                return G, E, last_exp

            def emit_ln_block(group_G, group_E, last_exp):
                for G, E in zip(group_G, group_E):
                    for dt_ in range(2):
                        ln = nc.scalar.activation(
                            G[:, dt_, :], E[:, dt_, :], AF.Ln, bias=1.0
                        )
                        add_dep_helper(
                            ln.ins, last_exp.ins, sync=False,
                            reason="batch ACT Ln block after Exp block",
                        )

            def emit_smm(t0, group_G, lpT):
                for i, G in enumerate(group_G):
                    t = t0 + i
                    for lt in range(2):
                        plp = plpp.tile([128, CHUNK], F32, tag="plp")
                        for k in range(4):
                            nc.tensor.matmul(
                                plp[:, :],
                                smat_sb[:, k, lt * 128 : (lt + 1) * 128],
                                G[:, k, :],
                                start=(k == 0),
                                stop=(k == 3),
                            )
                        nc.scalar.activation(
                            lpT[:, t, lt, :], plp[:, :], AF.Exp, scale=-1.0
                        )

            def emit_mm2(ci, lpT):
                c0 = ci * CHUNK
                for s in range(CHUNK // 128):
                    po = pop.tile([128, 1024], F32, tag="po")
                    n_acc = n_trees * 2
                    i = 0
                    for t in range(n_trees):
                        for lt in range(2):
                            first = i == 0
                            last = i == n_acc - 1
                            lhsT = lpT[:, t, lt, s * 128 : (s + 1) * 128]
                            nc.tensor.matmul(
                                po[:, 0:500], lhsT, w2_sb[:, t, lt, 0:500],
                                start=first, stop=last,
                            )
                            nc.tensor.matmul(
                                po[:, 512:1012], lhsT, w2_sb[:, t, lt, 500:1000],
                                start=first, stop=last,
                            )
                            i += 1
                    osb = outp.tile([128, CLASSES], F32, tag="osb")
                    nc.vector.tensor_copy(osb[:, 0:500], po[:, 0:500])
                    nc.vector.tensor_copy(osb[:, 500:1000], po[:, 512:1012])
                    nc.sync.dma_start(
                        out[c0 + s * 128 : c0 + (s + 1) * 128, :], osb[:, :]
                    )

            for ci in range(n_chunks):
                xp = xt_tiles[ci]
                lpT = lptp.tile([128, n_trees, 2, CHUNK], F16, tag="lpT")
                # software pipeline: the S-block of group g is emitted after
                # two trees of group g+1 (far enough for the Exp->Ln latency,
                # close enough that G-tile reuse (gp bufs=5) can't deadlock)
                pend = None  # (t0, group_G) awaiting S-matmuls
                for t0 in range(0, n_trees, GROUP):
                    group = list(range(t0, min(t0 + GROUP, n_trees)))
                    group_G, group_E, last_exp = [], [], None
                    for i, t in enumerate(group):
                        G, E, last_exp = emit_decision(t, xp, lpT)
                        group_G.append(G)
                        group_E.append(E)
                        if pend is not None and i == 1:
                            emit_smm(*pend, lpT)
                            pend = None
                    emit_ln_block(group_G, group_E, last_exp)
                    pend = (t0, group_G)
                # prefetch next chunk's x while S/emit_mm2 fill the PE
                if ci + 1 < n_chunks:
                    load_x(ci + 1)
                emit_smm(*pend, lpT)
                emit_mm2(ci, lpT)
    nc.compile()
    return nc


def _smat_np():
    # rows 0:256 multiply softplus(-z) (path indicator); rows 256:511 multiply
    # the raw psum (1024*z), so carry the 1/1024 descale here.
    S = np.zeros((512, NQ), np.float32)
    q = np.arange(NQ)
    for n in range(8):
        node = (2**n - 1) + (q >> (8 - n))
        branch = (q >> (7 - n)) & 1
        S[node, q] += 1.0
        S[256 + node, q] += branch.astype(np.float32) * np.float32(Z_DESCALE)
    # pre-tiled [128, 4, NQ]
    return np.ascontiguousarray(S.reshape(4, 128, NQ).transpose(1, 0, 2))


def _prep_weights(w_d, w_l, n_trees=N_TREES):
    fp8 = ml_dtypes.float8_e4m3
    w_l = np.asarray(w_l, dtype=np.float32)
    m = w_l.max(axis=-1, keepdims=True)
    e = np.exp(w_l - m, dtype=np.float32)
    sm = e / e.sum(axis=-1, keepdims=True)
    w2 = (sm[:, 0::2, :] + sm[:, 1::2, :]) * np.float32(1.0 / n_trees)
    # [t, 256, C] -> pre-tiled [t, 128, 2, C]
    w2 = np.ascontiguousarray(
        w2.reshape(n_trees, 2, 128, CLASSES).transpose(0, 2, 1, 3)
    ).astype(np.float16)
    wd_p = np.zeros((n_trees, IN_DIM, ND_PAD), np.float32)
    wd_p[:, :, : w_d.shape[2]] = w_d * np.float32(WD_SCALE)
    # [t, (k p), d] -> pre-tiled [t, 128, k, d], split fp8/f16 k ranges
    wd_t = wd_p.reshape(n_trees, KI, 128, ND_PAD).transpose(0, 2, 1, 3)
    wd_8 = np.ascontiguousarray(wd_t[:, :, :K8]).astype(fp8)
    wd_16 = np.ascontiguousarray(wd_t[:, :, K8:]).astype(np.float16)
    return wd_8, wd_16, _smat_np(), w2


last_bass_results = None


def kernel(x, w_d, w_l):
    global last_bass_results
    x = np.asarray(x)
    wd_8, wd_16, S, w2 = _prep_weights(np.asarray(w_d), np.asarray(w_l))
    xs = x * np.float32(X_SCALE)
    in_maps = []
    for c in range(N_CORES):
        # [b_loc, IN_DIM] -> [128, n_chunks, k, CHUNK]:
        # xt[p, ci, k, n] = xs[c*B_LOC + ci*CHUNK + n, k*128 + p]
        xc = xs[c * B_LOC : (c + 1) * B_LOC, :]
        xct = xc.reshape(N_CHUNKS, CHUNK, KI, 128).transpose(3, 0, 2, 1)
        x_8 = np.ascontiguousarray(xct[:, :, :K8]).astype(ml_dtypes.float8_e4m3)
        x_16 = np.ascontiguousarray(xct[:, :, K8:]).astype(np.float16)
        in_maps.append(
            {"xt8": x_8, "xt16": x_16, "wd8": wd_8, "wd16": wd_16,
             "smat": S, "w2": w2}
        )
    if "nc" not in _CACHE:
        _CACHE["nc"] = _build()
    res = run_bass_kernel_spmd(_CACHE["nc"], in_maps, core_ids=list(range(N_CORES)))
    last_bass_results = res
    return np.concatenate([res.results[c]["out"] for c in range(N_CORES)], axis=0)


# revision 9
# speedup vs baseline: 1.0586x; 1.0586x over previous
"""Trainium2 Bass kernel for nn_DiffForest (soft decision forest forward).

Math: per tree t, z = x @ w_d[t]; p = sigmoid(z); leaf path probs are products
of 8 factors p/(1-p) down a depth-8 tree; output = sum_t leaf_prob @ softmax(w_l[t]) / 10.

Kernel formulation (all on device except small weight prep):
  - The 512 "leaves" come in identical pairs -> fold to 256 paths; fold the
    pair-sum + 1/n_trees into the leaf weight matrix w2 (host, exact).
  - Path products move to log space:  -log P[q] = sum_path softplus(-z) + sum_{branch=1} z
    which is a matmul with a constant matrix S [512, 256]:
        A = S^T @ [softplus(-z); z],   leaf_prob^T = exp(-A)   ([256 paths, batch])
    softplus(-z) = ln(1 + exp(-z)) via the Exp/Ln activation tables.
  - Decision matmul: mixed precision. 12 of 16 k-tiles run fp8e4 (x scaled by
    16, w_d by 64) with MatmulPerfMode.DoubleRow (two k-tiles per pass, 2x PE
    throughput); the last 4 k-tiles run f16 at the same 1024 product scale and
    accumulate into the same PSUM. Full-fp8 measures 1.92e-2 rel err vs the
    2e-2 gate; the f16 fraction buys the error margin back (sqrt(12/16) x).
    The 1/1024 descale folds into the Exp activation scale and the z-half of S.
  - S-matmul in fp32r; leaf matmul in f16 (fp8 there costs ~1.2e-2 rel err).
  - All dram tensors are host-pre-tiled so every DMA moves long contiguous
    lines per partition (256B lines measured ~200GB/s; 4-8KB lines fix that).
  - The S-matmul block of group g is emitted after two trees of group g+1 so
    the in-order PE rides through the group's Exp->Ln->table-load latency.
  - Sharding: data-parallel over batch; each of the 8 cores takes 2048 rows of x,
    weights replicated, no collectives.
"""

import numpy as np
import ml_dtypes

import concourse.bacc as bacc
import concourse.mybir as mybir
import concourse.tile as tile
from concourse.tile import add_dep_helper
from concourse.bass_utils import run_bass_kernel_spmd

N_CORES = 8
BATCH = 16384
B_LOC = BATCH // N_CORES        # 2048 rows per core
IN_DIM = 2048
N_TREES = 10
ND_PAD = 256                    # decision nodes padded 255 -> 256
NQ = 256                        # folded path (leaf) count
CLASSES = 1000
CHUNK = 512                     # batch columns processed per chunk
KI = IN_DIM // 128              # 16 contraction tiles for the decision matmul
K8 = 12                         # k-tiles in fp8 (DoubleRow pairs)
K16 = KI - K8                   # k-tiles in f16
N_CHUNKS = B_LOC // CHUNK

BF16 = mybir.dt.bfloat16
F32 = mybir.dt.float32
F32R = mybir.dt.float32r
F16 = mybir.dt.float16
F8 = mybir.dt.float8e4
AF = mybir.ActivationFunctionType
DR = mybir.MatmulPerfMode.DoubleRow

X_SCALE = 16.0                  # x -> fp8/f16
WD_SCALE = 64.0                 # w_d -> fp8/f16
Z_DESCALE = 1.0 / (X_SCALE * WD_SCALE)   # psum holds 1024*z

_CACHE = {}


def _build(b_loc=B_LOC, n_trees=N_TREES):
    n_chunks = b_loc // CHUNK
    nc = bacc.Bacc("TRN2", target_bir_lowering=False)
    # host-pre-tiled layouts: partition dim first, contiguous k*free lines
    xt8 = nc.dram_tensor("xt8", (128, n_chunks, K8, CHUNK), F8, kind="ExternalInput")
    xt16 = nc.dram_tensor(
        "xt16", (128, n_chunks, K16, CHUNK), F16, kind="ExternalInput"
    )
    wd8 = nc.dram_tensor("wd8", (n_trees, 128, K8, ND_PAD), F8, kind="ExternalInput")
    wd16 = nc.dram_tensor(
        "wd16", (n_trees, 128, K16, ND_PAD), F16, kind="ExternalInput"
    )
    smat = nc.dram_tensor("smat", (128, 4, NQ), F32R, kind="ExternalInput")
    w2 = nc.dram_tensor("w2", (n_trees, 128, 2, CLASSES), F16, kind="ExternalInput")
    out = nc.dram_tensor("out", (b_loc, CLASSES), F32, kind="ExternalOutput")

    with tile.TileContext(nc) as tc:
        with (
            tc.tile_pool(name="const", bufs=1) as constp,
            tc.tile_pool(name="sb", bufs=2) as sb,
            tc.tile_pool(name="ep", bufs=5) as ep,
            tc.tile_pool(name="gp", bufs=6) as gp,
            tc.tile_pool(name="outp", bufs=2) as outp,
            tc.tile_pool(name="lptp", bufs=1) as lptp,
            tc.tile_pool(name="pz", bufs=2, space="PSUM") as pzp,
            tc.tile_pool(name="plp", bufs=2, space="PSUM") as plpp,
            tc.tile_pool(name="po", bufs=2, space="PSUM") as pop,
        ):
            smat_sb = constp.tile([128, 4, NQ], F32R)
            w2_sb = constp.tile([128, n_trees, 2, CLASSES], F16)

            GROUP = 5
            first_mm = [None]

            # chunk-0 x load first: it heads the critical path
            xt_tiles = [None] * n_chunks

            def load_x(ci, split=False):
                xa = sb.tile([128, K8, CHUNK], F8, tag="xt8")
                xb = sb.tile([128, K16, CHUNK], F16, tag="xt16")
                if not split:
                    nc.sync.dma_start(xa[:, :, :], xt8[:, ci, :, :])
                    nc.sync.dma_start(xb[:, :, :], xt16[:, ci, :, :])
                xt_tiles[ci] = (xa, xb)
                return xa, xb

            wd_tiles = [None] * n_trees

            def load_wd(t, split=False):
                wa = constp.tile([128, K8, ND_PAD], F8, tag=f"wd8_{t}")
                wb = constp.tile([128, K16, ND_PAD], F16, tag=f"wd16_{t}")
                dmas = []
                if not split:
                    dmas.append(nc.sync.dma_start(wa[:, :, :], wd8[t, :, :, :]))
                    dmas.append(nc.sync.dma_start(wb[:, :, :], wd16[t, :, :, :]))
                wd_tiles[t] = (wa, wb)
                return dmas

            # startup: interleave split x/wd pieces so the first DR matmul
            # (reading k-tiles 0:2) waits on the minimum number of bytes
            xa0, xb0 = load_x(0, split=True)
            load_wd(0, split=True)
            wa0, wb0 = wd_tiles[0]
            h = K8 // 2
            nc.sync.dma_start(xa0[:, 0:h, :], xt8[:, 0, 0:h, :])
            nc.sync.dma_start(wa0[:, 0:h, :], wd8[0, :, 0:h, :])
            nc.sync.dma_start(xa0[:, h:K8, :], xt8[:, 0, h:K8, :])
            nc.sync.dma_start(wa0[:, h:K8, :], wd8[0, :, h:K8, :])
            nc.sync.dma_start(xb0[:, :, :], xt16[:, 0, :, :])
            nc.sync.dma_start(wb0[:, :, :], wd16[0, :, :, :])

            def emit_deferred():
                # behind the first matmul so they can't crowd the startup queues
                dmas = [nc.sync.dma_start(smat_sb[:, :, :], smat[:, :, :])]
                for t in range(1, n_trees):
                    dmas.extend(load_wd(t))

                for t in range(n_trees):
                    dmas.append(
                        nc.sync.dma_start(w2_sb[:, t, :, :], w2[t, :, :, :])
                    )
                for dma in dmas:
                    add_dep_helper(
                        dma.ins, first_mm[0].ins, sync=True,
                        reason="startup: critical pieces first",
                    )

            def emit_decision(t, xp, lpT):
                """mixed fp8-DoubleRow / f16 decision matmuls + Exp/z-copy."""
                xa, xb = xp
                wa, wb = wd_tiles[t]
                G = gp.tile([128, 4, CHUNK], F32R, tag="G")
                E = ep.tile([128, 2, CHUNK], F16, tag="E")
                last_exp = None
                fresh = first_mm[0] is None
                for dt_ in range(2):
                    psz = pzp.tile([128, CHUNK], F32, tag="psz")
                    for j in range(K8 // 2):
                        mm = nc.tensor.matmul(
                            psz[:, :],
                            wa[:, 2 * j : 2 * j + 2, dt_ * 128 : (dt_ + 1) * 128],
                            xa[:, 2 * j : 2 * j + 2, :],
                            start=(j == 0),
                            stop=False,
                            perf_mode=DR,
                        )
                        if first_mm[0] is None:
                            first_mm[0] = mm
                    for j in range(K16):
                        nc.tensor.matmul(
                            psz[:, :],
                            wb[:, j, dt_ * 128 : (dt_ + 1) * 128],
                            xb[:, j, :],
                            start=False,
                            stop=(j == K16 - 1),
                        )
                    # DVE copy is the ONLY psz reader (fast PSUM release);
                    # Exp reads the SBUF copy so the ACT queue's Ln-block +
                    # table-load latency can't block the next PE chain.
                    nc.vector.tensor_copy(G[:, 2 + dt_, :], psz[:, :])
                    last_exp = nc.scalar.activation(
                        E[:, dt_, :], G[:, 2 + dt_, :], AF.Exp, scale=-Z_DESCALE
                    )
                if fresh:
                    emit_deferred()
                return G, E, last_exp

            def emit_ln_block(group_G, group_E, last_exp):
                for G, E in zip(group_G, group_E):
                    for dt_ in range(2):
                        ln = nc.scalar.activation(
                            G[:, dt_, :], E[:, dt_, :], AF.Ln, bias=1.0
                        )
                        add_dep_helper(
                            ln.ins, last_exp.ins, sync=False,
                            reason="batch ACT Ln block after Exp block",
                        )

            def emit_smm(t0, group_G, lpT):
                for i, G in enumerate(group_G):
                    t = t0 + i
                    for lt in range(2):
                        plp = plpp.tile([128, CHUNK], F32, tag="plp")
                        for k in range(4):
                            nc.tensor.matmul(
                                plp[:, :],
                                smat_sb[:, k, lt * 128 : (lt + 1) * 128],
                                G[:, k, :],
                                start=(k == 0),
                                stop=(k == 3),
                            )
                        nc.scalar.activation(
                            lpT[:, t, lt, :], plp[:, :], AF.Exp, scale=-1.0
                        )

            def emit_mm2(ci, lpT):
                c0 = ci * CHUNK
                for s in range(CHUNK // 128):
                    po = pop.tile([128, 1024], F32, tag="po")
                    n_acc = n_trees * 2
                    i = 0
                    for t in range(n_trees):
                        for lt in range(2):
                            first = i == 0
                            last = i == n_acc - 1
                            lhsT = lpT[:, t, lt, s * 128 : (s + 1) * 128]
                            nc.tensor.matmul(
                                po[:, 0:500], lhsT, w2_sb[:, t, lt, 0:500],
                                start=first, stop=last,
                            )
                            nc.tensor.matmul(
                                po[:, 512:1012], lhsT, w2_sb[:, t, lt, 500:1000],
                                start=first, stop=last,
                            )
                            i += 1
                    osb = outp.tile([128, CLASSES], F32, tag="osb")
                    nc.vector.tensor_copy(osb[:, 0:500], po[:, 0:500])
                    nc.vector.tensor_copy(osb[:, 500:1000], po[:, 512:1012])
                    nc.sync.dma_start(
                        out[c0 + s * 128 : c0 + (s + 1) * 128, :], osb[:, :]
                    )

            for ci in range(n_chunks):
                xp = xt_tiles[ci]
                lpT = lptp.tile([128, n_trees, 2, CHUNK], F16, tag="lpT")
                # software pipeline: the S-block of group g is emitted after
                # two trees of group g+1 (far enough for the Exp->Ln latency,
                # close enough that G-tile reuse (gp bufs=5) can't deadlock)
                pend = None  # (t0, group_G) awaiting S-matmuls
                for t0 in range(0, n_trees, GROUP):
                    group = list(range(t0, min(t0 + GROUP, n_trees)))
                    group_G, group_E, last_exp = [], [], None
                    for i, t in enumerate(group):
                        G, E, last_exp = emit_decision(t, xp, lpT)
                        group_G.append(G)
                        group_E.append(E)
                        if pend is not None and i == 1:
                            emit_smm(*pend, lpT)
                            pend = None
                    emit_ln_block(group_G, group_E, last_exp)
                    pend = (t0, group_G)
                # prefetch next chunk's x while S/emit_mm2 fill the PE
                if ci + 1 < n_chunks:
                    load_x(ci + 1)
                emit_smm(*pend, lpT)
                emit_mm2(ci, lpT)
    nc.compile()
    return nc


def _smat_np():
    # rows 0:256 multiply softplus(-z) (path indicator); rows 256:511 multiply
    # the raw psum (1024*z), so carry the 1/1024 descale here.
    S = np.zeros((512, NQ), np.float32)
    q = np.arange(NQ)
    for n in range(8):
        node = (2**n - 1) + (q >> (8 - n))
        branch = (q >> (7 - n)) & 1
        S[node, q] += 1.0
        S[256 + node, q] += branch.astype(np.float32) * np.float32(Z_DESCALE)
    # pre-tiled [128, 4, NQ]
    return np.ascontiguousarray(S.reshape(4, 128, NQ).transpose(1, 0, 2))


def _prep_weights(w_d, w_l, n_trees=N_TREES):
    fp8 = ml_dtypes.float8_e4m3
    w_l = np.asarray(w_l, dtype=np.float32)
    m = w_l.max(axis=-1, keepdims=True)
    e = np.exp(w_l - m, dtype=np.float32)
    sm = e / e.sum(axis=-1, keepdims=True)
    w2 = (sm[:, 0::2, :] + sm[:, 1::2, :]) * np.float32(1.0 / n_trees)
    # [t, 256, C] -> pre-tiled [t, 128, 2, C]
    w2 = np.ascontiguousarray(
        w2.reshape(n_trees, 2, 128, CLASSES).transpose(0, 2, 1, 3)
    ).astype(np.float16)
    wd_p = np.zeros((n_trees, IN_DIM, ND_PAD), np.float32)
    wd_p[:, :, : w_d.shape[2]] = w_d * np.float32(WD_SCALE)
    # [t, (k p), d] -> pre-tiled [t, 128, k, d], split fp8/f16 k ranges
    wd_t = wd_p.reshape(n_trees, KI, 128, ND_PAD).transpose(0, 2, 1, 3)
    wd_8 = np.ascontiguousarray(wd_t[:, :, :K8]).astype(fp8)
    wd_16 = np.ascontiguousarray(wd_t[:, :, K8:]).astype(np.float16)
    return wd_8, wd_16, _smat_np(), w2


last_bass_results = None


def kernel(x, w_d, w_l):
    global last_bass_results
    x = np.asarray(x)
    wd_8, wd_16, S, w2 = _prep_weights(np.asarray(w_d), np.asarray(w_l))
    xs = x * np.float32(X_SCALE)
    in_maps = []
    for c in range(N_CORES):
        # [b_loc, IN_DIM] -> [128, n_chunks, k, CHUNK]:
        # xt[p, ci, k, n] = xs[c*B_LOC + ci*CHUNK + n, k*128 + p]
        xc = xs[c * B_LOC : (c + 1) * B_LOC, :]
        xct = xc.reshape(N_CHUNKS, CHUNK, KI, 128).transpose(3, 0, 2, 1)
        x_8 = np.ascontiguousarray(xct[:, :, :K8]).astype(ml_dtypes.float8_e4m3)
        x_16 = np.ascontiguousarray(xct[:, :, K8:]).astype(np.float16)
        in_maps.append(
            {"xt8": x_8, "xt16": x_16, "wd8": wd_8, "wd16": wd_16,
             "smat": S, "w2": w2}
        )
    if "nc" not in _CACHE:
        _CACHE["nc"] = _build()
    res = run_bass_kernel_spmd(_CACHE["nc"], in_maps, core_ids=list(range(N_CORES)))
    last_bass_results = res
    return np.concatenate([res.results[c]["out"] for c in range(N_CORES)], axis=0)


# revision 12
# speedup vs baseline: 1.1864x; 1.1208x over previous
"""Trainium2 Bass kernel for nn_DiffForest (soft decision forest forward).

Math: per tree t, z = x @ w_d[t]; p = sigmoid(z); leaf path probs are products
of 8 factors p/(1-p) down a depth-8 tree; output = sum_t leaf_prob @ softmax(w_l[t]) / 10.

Kernel formulation (all on device except small weight prep):
  - The 512 "leaves" come in identical pairs -> fold to 256 paths; fold the
    pair-sum + 1/n_trees into the leaf weight matrix w2 (host, exact).
  - Path products move to log space:  -log P[q] = sum_path softplus(-z) + sum_{branch=1} z
    which is a matmul with a constant matrix S [512, 256]:
        A = S^T @ [softplus(-z); z],   leaf_prob^T = exp(-A)   ([256 paths, batch])
    softplus(-z) = ln(1 + exp(-z)) via the Exp/Ln activation tables.
  - Decision matmul: mixed precision. 12 of 16 k-tiles run fp8e4 (x scaled by
    16, w_d by 64) with MatmulPerfMode.DoubleRow (two k-tiles per pass, 2x PE
    throughput); the last 4 k-tiles run f16 at the same 1024 product scale and
    accumulate into the same PSUM. Full-fp8 measures 1.92e-2 rel err vs the
    2e-2 gate; the f16 fraction buys the error margin back (sqrt(12/16) x).
    The 1/1024 descale folds into the Exp activation scale and the z-half of S.
  - S-matmul in fp32r; leaf matmul in f16 (fp8 there costs ~1.2e-2 rel err).
  - All dram tensors are host-pre-tiled so every DMA moves long contiguous
    lines per partition (256B lines measured ~200GB/s; 4-8KB lines fix that).
  - The S-matmul block of group g is emitted after two trees of group g+1 so
    the in-order PE rides through the group's Exp->Ln->table-load latency.
  - Sharding: data-parallel over batch; each of the 8 cores takes 2048 rows of x,
    weights replicated, no collectives.
"""

import numpy as np
import ml_dtypes

import concourse.bacc as bacc
import concourse.mybir as mybir
import concourse.tile as tile
from concourse.tile import add_dep_helper
from concourse.bass_utils import run_bass_kernel_spmd

N_CORES = 8
BATCH = 16384
B_LOC = BATCH // N_CORES        # 2048 rows per core
IN_DIM = 2048
N_TREES = 10
ND_PAD = 256                    # decision nodes padded 255 -> 256
NQ = 256                        # folded path (leaf) count
CLASSES = 1000
CHUNK = 512                     # batch columns processed per chunk
KI = IN_DIM // 128              # 16 contraction tiles for the decision matmul
K8 = 12                         # k-tiles in fp8 (DoubleRow pairs)
K16 = KI - K8                   # k-tiles in f16
N_CHUNKS = B_LOC // CHUNK

BF16 = mybir.dt.bfloat16
F32 = mybir.dt.float32
F32R = mybir.dt.float32r
F16 = mybir.dt.float16
F8 = mybir.dt.float8e4
AF = mybir.ActivationFunctionType
DR = mybir.MatmulPerfMode.DoubleRow

X_SCALE = 16.0                  # x -> fp8/f16
WD_SCALE = 64.0                 # w_d -> fp8/f16
Z_DESCALE = 1.0 / (X_SCALE * WD_SCALE)   # psum holds 1024*z

_CACHE = {}


def _build(b_loc=B_LOC, n_trees=N_TREES):
    n_chunks = b_loc // CHUNK
    nc = bacc.Bacc("TRN2", target_bir_lowering=False)
    # host-pre-tiled layouts: partition dim first, contiguous k*free lines
    xt8 = nc.dram_tensor("xt8", (128, n_chunks, K8, CHUNK), F8, kind="ExternalInput")
    xt16 = nc.dram_tensor(
        "xt16", (128, n_chunks, K16, CHUNK), F16, kind="ExternalInput"
    )
    wd8 = nc.dram_tensor("wd8", (n_trees, 128, K8, ND_PAD), F8, kind="ExternalInput")
    wd16 = nc.dram_tensor(
        "wd16", (n_trees, 128, K16, ND_PAD), F16, kind="ExternalInput"
    )
    smat = nc.dram_tensor("smat", (128, 4, 128), F32R, kind="ExternalInput")
    w2 = nc.dram_tensor("w2", (n_trees, 128, 2, CLASSES), F16, kind="ExternalInput")
    out = nc.dram_tensor("out", (b_loc, CLASSES), F32, kind="ExternalOutput")

    with tile.TileContext(nc) as tc:
        with (
            tc.tile_pool(name="const", bufs=1) as constp,
            tc.tile_pool(name="sb", bufs=2) as sb,
            tc.tile_pool(name="ep", bufs=5) as ep,
            tc.tile_pool(name="gp", bufs=6) as gp,
            tc.tile_pool(name="outp", bufs=2) as outp,
            tc.tile_pool(name="lptp", bufs=1) as lptp,
            tc.tile_pool(name="pz", bufs=2, space="PSUM") as pzp,
            tc.tile_pool(name="plp", bufs=2, space="PSUM") as plpp,
            tc.tile_pool(name="po", bufs=2, space="PSUM") as pop,
        ):
            smat_sb = constp.tile([128, 4, 128], F32R)
            w2_sb = constp.tile([128, n_trees, 2, CLASSES], F16)

            GROUP = 5
            first_mm = [None]

            # chunk-0 x load first: it heads the critical path
            xt_tiles = [None] * n_chunks

            def load_x(ci, split=False):
                xa = sb.tile([128, K8, CHUNK], F8, tag="xt8")
                xb = sb.tile([128, K16, CHUNK], F16, tag="xt16")
                if not split:
                    nc.sync.dma_start(xa[:, :, :], xt8[:, ci, :, :])
                    nc.sync.dma_start(xb[:, :, :], xt16[:, ci, :, :])
                xt_tiles[ci] = (xa, xb)
                return xa, xb

            wd_tiles = [None] * n_trees

            def load_wd(t, split=False):
                wa = constp.tile([128, K8, ND_PAD], F8, tag=f"wd8_{t}")
                wb = constp.tile([128, K16, ND_PAD], F16, tag=f"wd16_{t}")
                dmas = []
                if not split:
                    dmas.append(nc.sync.dma_start(wa[:, :, :], wd8[t, :, :, :]))
                    dmas.append(nc.sync.dma_start(wb[:, :, :], wd16[t, :, :, :]))
                wd_tiles[t] = (wa, wb)
                return dmas

            # startup: interleave split x/wd pieces so the first DR matmul
            # (reading k-tiles 0:2) waits on the minimum number of bytes
            xa0, xb0 = load_x(0, split=True)
            load_wd(0, split=True)
            wa0, wb0 = wd_tiles[0]
            h = K8 // 2
            nc.sync.dma_start(xa0[:, 0:h, :], xt8[:, 0, 0:h, :])
            nc.sync.dma_start(wa0[:, 0:h, :], wd8[0, :, 0:h, :])
            nc.sync.dma_start(xa0[:, h:K8, :], xt8[:, 0, h:K8, :])
            nc.sync.dma_start(wa0[:, h:K8, :], wd8[0, :, h:K8, :])
            nc.sync.dma_start(xb0[:, :, :], xt16[:, 0, :, :])
            nc.sync.dma_start(wb0[:, :, :], wd16[0, :, :, :])

            def emit_deferred():
                # behind the first matmul so they can't crowd the startup queues
                dmas = [nc.sync.dma_start(smat_sb[:, :, :], smat[:, :, :])]
                for t in range(1, n_trees):
                    dmas.extend(load_wd(t))

                for t in range(n_trees):
                    dmas.append(
                        nc.sync.dma_start(w2_sb[:, t, :, :], w2[t, :, :, :])
                    )
                for dma in dmas:
                    add_dep_helper(
                        dma.ins, first_mm[0].ins, sync=True,
                        reason="startup: critical pieces first",
                    )

            def emit_decision(t, xp, lpT):
                """mixed fp8-DoubleRow / f16 decision matmuls + Exp/z-copy."""
                xa, xb = xp
                wa, wb = wd_tiles[t]
                G = gp.tile([128, 4, CHUNK], F32R, tag="G")
                E = ep.tile([128, 2, CHUNK], F16, tag="E")
                last_exp = None
                fresh = first_mm[0] is None
                for dt_ in range(2):
                    psz = pzp.tile([128, CHUNK], F32, tag="psz")
                    for j in range(K8 // 2):
                        mm = nc.tensor.matmul(
                            psz[:, :],
                            wa[:, 2 * j : 2 * j + 2, dt_ * 128 : (dt_ + 1) * 128],
                            xa[:, 2 * j : 2 * j + 2, :],
                            start=(j == 0),
                            stop=False,
                            perf_mode=DR,
                        )
                        if first_mm[0] is None:
                            first_mm[0] = mm
                    for j in range(K16):
                        nc.tensor.matmul(
                            psz[:, :],
                            wb[:, j, dt_ * 128 : (dt_ + 1) * 128],
                            xb[:, j, :],
                            start=False,
                            stop=(j == K16 - 1),
                        )
                    # DVE copy is the ONLY psz reader (fast PSUM release);
                    # Exp reads the SBUF copy so the ACT queue's Ln-block +
                    # table-load latency can't block the next PE chain.
                    nc.vector.tensor_copy(G[:, 2 + dt_, :], psz[:, :])
                # one paired Exp over both halves (fewer ACT fixed overheads)
                last_exp = nc.scalar.activation(
                    E[:, :, :], G[:, 2:4, :], AF.Exp, scale=-Z_DESCALE
                )
                if fresh:
                    emit_deferred()
                return G, E, last_exp

            def emit_ln_block(group_G, group_E, last_exp):
                for G, E in zip(group_G, group_E):
                    ln = nc.scalar.activation(
                        G[:, 0:2, :], E[:, :, :], AF.Ln, bias=1.0
                    )
                    add_dep_helper(
                        ln.ins, last_exp.ins, sync=False,
                        reason="batch ACT Ln block after Exp block",
                    )

            def emit_smm(t0, group_G, lpT):
                # node-permuted S: leaf-half lt only needs node-tile lt, so
                # each half is a 2-deep accumulation (sp + z) instead of 4
                for i, G in enumerate(group_G):
                    t = t0 + i
                    plp = plpp.tile([128, 2, CHUNK], F32, tag="plp")
                    for lt in range(2):
                        nc.tensor.matmul(
                            plp[:, lt, :], smat_sb[:, lt, :], G[:, lt, :],
                            start=True, stop=False,
                        )
                        nc.tensor.matmul(
                            plp[:, lt, :], smat_sb[:, 2 + lt, :],
                            G[:, 2 + lt, :],
                            start=False, stop=True,
                        )
                    nc.scalar.activation(
                        lpT[:, t, 0:2, :], plp[:, :, :], AF.Exp, scale=-1.0
                    )

            def emit_mm2(ci, lpT):
                c0 = ci * CHUNK
                n_acc = n_trees * 2
                for s in range(CHUNK // 128):
                    osb = outp.tile([128, CLASSES], F32, tag="osb")
                    for half in range(2):
                        po = pop.tile([128, 512], F32, tag="po")
                        i = 0
                        for t in range(n_trees):
                            for lt in range(2):
                                nc.tensor.matmul(
                                    po[:, 0:500],
                                    lpT[:, t, lt, s * 128 : (s + 1) * 128],
                                    w2_sb[
                                        :, t, lt, half * 500 : half * 500 + 500
                                    ],
                                    start=(i == 0), stop=(i == n_acc - 1),
                                )
                                i += 1
                        nc.vector.tensor_copy(
                            osb[:, half * 500 : half * 500 + 500], po[:, 0:500]
                        )
                    nc.sync.dma_start(
                        out[c0 + s * 128 : c0 + (s + 1) * 128, :], osb[:, :]
                    )

            for ci in range(n_chunks):
                xp = xt_tiles[ci]
                lpT = lptp.tile([128, n_trees, 2, CHUNK], F16, tag="lpT")
                # software pipeline: the S-block of group g is emitted after
                # two trees of group g+1 (far enough for the Exp->Ln latency,
                # close enough that G-tile reuse (gp bufs=5) can't deadlock)
                pend = None  # (t0, group_G) awaiting S-matmuls
                for t0 in range(0, n_trees, GROUP):
                    group = list(range(t0, min(t0 + GROUP, n_trees)))
                    group_G, group_E, last_exp = [], [], None
                    for i, t in enumerate(group):
                        G, E, last_exp = emit_decision(t, xp, lpT)
                        group_G.append(G)
                        group_E.append(E)
                        if pend is not None and i == 1:
                            emit_smm(*pend, lpT)
                            pend = None
                    emit_ln_block(group_G, group_E, last_exp)
                    pend = (t0, group_G)
                # prefetch next chunk's x while S/emit_mm2 fill the PE
                if ci + 1 < n_chunks:
                    load_x(ci + 1)
                emit_smm(*pend, lpT)
                emit_mm2(ci, lpT)
    nc.compile()
    return nc


def _node_perm():
    # node-tile lt holds exactly the path nodes of leaf-half lt: the shared
    # root (node 0) is duplicated into the otherwise-unused 256th slot.
    p0 = [0, 1] + [2**n - 1 + j for n in range(2, 8) for j in range(2 ** (n - 1))]
    p1 = [0, 2] + [
        2**n - 1 + j for n in range(2, 8) for j in range(2 ** (n - 1), 2**n)
    ]
    return [p0, p1]


def _smat_np():
    # k=lt: path indicator (multiplies softplus(-z)); k=2+lt: branch bits
    # scaled by 1/1024 (multiplies the raw psum = 1024*z).
    perm = _node_perm()
    S = np.zeros((128, 4, 128), np.float32)
    for lt in range(2):
        inv = {node: p for p, node in enumerate(perm[lt])}
        for ql in range(128):
            q = lt * 128 + ql
            for n in range(8):
                node = 2**n - 1 + (q >> (8 - n))
                b = (q >> (7 - n)) & 1
                S[inv[node], lt, ql] += 1.0
                S[inv[node], 2 + lt, ql] += b * np.float32(Z_DESCALE)
    return S


def _prep_weights(w_d, w_l, n_trees=N_TREES):
    fp8 = ml_dtypes.float8_e4m3
    w_l = np.asarray(w_l, dtype=np.float32)
    m = w_l.max(axis=-1, keepdims=True)
    e = np.exp(w_l - m, dtype=np.float32)
    sm = e / e.sum(axis=-1, keepdims=True)
    w2 = (sm[:, 0::2, :] + sm[:, 1::2, :]) * np.float32(1.0 / n_trees)
    # [t, 256, C] -> pre-tiled [t, 128, 2, C]
    w2 = np.ascontiguousarray(
        w2.reshape(n_trees, 2, 128, CLASSES).transpose(0, 2, 1, 3)
    ).astype(np.float16)
    wd_p = np.zeros((n_trees, IN_DIM, ND_PAD), np.float32)
    wd_p[:, :, : w_d.shape[2]] = w_d * np.float32(WD_SCALE)
    # permute decision-node columns so node-tile lt serves leaf-half lt
    perm = np.array(_node_perm()).reshape(-1)
    wd_p = wd_p[:, :, perm]
    # [t, (k p), d] -> pre-tiled [t, 128, k, d], split fp8/f16 k ranges
    wd_t = wd_p.reshape(n_trees, KI, 128, ND_PAD).transpose(0, 2, 1, 3)
    wd_8 = np.ascontiguousarray(wd_t[:, :, :K8]).astype(fp8)
    wd_16 = np.ascontiguousarray(wd_t[:, :, K8:]).astype(np.float16)
    return wd_8, wd_16, _smat_np(), w2


last_bass_results = None


def kernel(x, w_d, w_l):
    global last_bass_results
    x = np.asarray(x)
    wd_8, wd_16, S, w2 = _prep_weights(np.asarray(w_d), np.asarray(w_l))
    xs = x * np.float32(X_SCALE)
    in_maps = []
    for c in range(N_CORES):
        # [b_loc, IN_DIM] -> [128, n_chunks, k, CHUNK]:
        # xt[p, ci, k, n] = xs[c*B_LOC + ci*CHUNK + n, k*128 + p]
        xc = xs[c * B_LOC : (c + 1) * B_LOC, :]
        xct = xc.reshape(N_CHUNKS, CHUNK, KI, 128).transpose(3, 0, 2, 1)
        x_8 = np.ascontiguousarray(xct[:, :, :K8]).astype(ml_dtypes.float8_e4m3)
        x_16 = np.ascontiguousarray(xct[:, :, K8:]).astype(np.float16)
        in_maps.append(
            {"xt8": x_8, "xt16": x_16, "wd8": wd_8, "wd16": wd_16,
             "smat": S, "w2": w2}
        )
    if "nc" not in _CACHE:
        _CACHE["nc"] = _build()
    res = run_bass_kernel_spmd(_CACHE["nc"], in_maps, core_ids=list(range(N_CORES)))
    last_bass_results = res
    return np.concatenate([res.results[c]["out"] for c in range(N_CORES)], axis=0)


# revision 14
# speedup vs baseline: 1.2066x; 1.0170x over previous
"""Trainium2 Bass kernel for nn_DiffForest (soft decision forest forward).

Math: per tree t, z = x @ w_d[t]; p = sigmoid(z); leaf path probs are products
of 8 factors p/(1-p) down a depth-8 tree; output = sum_t leaf_prob @ softmax(w_l[t]) / 10.

Kernel formulation (all on device except small weight prep):
  - The 512 "leaves" come in identical pairs -> fold to 256 paths; fold the
    pair-sum + 1/n_trees into the leaf weight matrix w2 (host, exact).
  - Path products move to log space:  -log P[q] = sum_path softplus(-z) + sum_{branch=1} z
    which is a matmul with a constant matrix S [512, 256]:
        A = S^T @ [softplus(-z); z],   leaf_prob^T = exp(-A)   ([256 paths, batch])
    softplus(-z) = ln(1 + exp(-z)) via the Exp/Ln activation tables.
  - Decision matmul: mixed precision. 14 of 16 k-tiles run fp8e4 (x scaled by
    16, w_d by 64) with MatmulPerfMode.DoubleRow (two k-tiles per pass, 2x PE
    throughput); the last 2 k-tiles run f16 at the same 1024 product scale and
    accumulate into the same PSUM. Full-fp8 measures 1.95e-2 rel err vs the
    2e-2 gate; the f16 fraction buys the error margin back.
    The 1/1024 descale folds into the Exp activation scale and the z-half of S.
  - S-matmul in fp32r; leaf matmul in f16 (fp8 there costs ~1.2e-2 rel err).
  - All dram tensors are host-pre-tiled so every DMA moves long contiguous
    lines per partition (256B lines measured ~200GB/s; 4-8KB lines fix that).
  - The S-matmul block of group g is emitted after two trees of group g+1 so
    the in-order PE rides through the group's Exp->Ln->table-load latency.
  - Sharding: data-parallel over batch; each of the 8 cores takes 2048 rows of x,
    weights replicated, no collectives.
"""

import numpy as np
import ml_dtypes

import concourse.bacc as bacc
import concourse.mybir as mybir
import concourse.tile as tile
from concourse.tile import add_dep_helper
from concourse.bass_utils import run_bass_kernel_spmd

N_CORES = 8
BATCH = 16384
B_LOC = BATCH // N_CORES        # 2048 rows per core
IN_DIM = 2048
N_TREES = 10
ND_PAD = 256                    # decision nodes padded 255 -> 256
NQ = 256                        # folded path (leaf) count
CLASSES = 1000
CHUNK = 512                     # batch columns processed per chunk
KI = IN_DIM // 128              # 16 contraction tiles for the decision matmul
K8 = 14                         # k-tiles in fp8 (DoubleRow pairs)
K16 = KI - K8                   # k-tiles in f16
N_CHUNKS = B_LOC // CHUNK

BF16 = mybir.dt.bfloat16
F32 = mybir.dt.float32
F32R = mybir.dt.float32r
F16 = mybir.dt.float16
F8 = mybir.dt.float8e4
AF = mybir.ActivationFunctionType
DR = mybir.MatmulPerfMode.DoubleRow

X_SCALE = 16.0                  # x -> fp8/f16
WD_SCALE = 64.0                 # w_d -> fp8/f16
Z_DESCALE = 1.0 / (X_SCALE * WD_SCALE)   # psum holds 1024*z

_CACHE = {}


def _build(b_loc=B_LOC, n_trees=N_TREES):
    n_chunks = b_loc // CHUNK
    nc = bacc.Bacc("TRN2", target_bir_lowering=False)
    # host-pre-tiled layouts: partition dim first, contiguous k*free lines
    xt8 = nc.dram_tensor("xt8", (128, n_chunks, K8, CHUNK), F8, kind="ExternalInput")
    xt16 = nc.dram_tensor(
        "xt16", (128, n_chunks, K16, CHUNK), F16, kind="ExternalInput"
    )
    wd8 = nc.dram_tensor("wd8", (n_trees, 128, K8, ND_PAD), F8, kind="ExternalInput")
    wd16 = nc.dram_tensor(
        "wd16", (n_trees, 128, K16, ND_PAD), F16, kind="ExternalInput"
    )
    smat = nc.dram_tensor("smat", (128, 4, 128), F32R, kind="ExternalInput")
    w2 = nc.dram_tensor("w2", (n_trees, 128, 2, CLASSES), F16, kind="ExternalInput")
    out = nc.dram_tensor("out", (b_loc, CLASSES), F32, kind="ExternalOutput")

    with tile.TileContext(nc) as tc:
        with (
            tc.tile_pool(name="const", bufs=1) as constp,
            tc.tile_pool(name="sb", bufs=2) as sb,
            tc.tile_pool(name="ep", bufs=5) as ep,
            tc.tile_pool(name="gp", bufs=6) as gp,
            tc.tile_pool(name="outp", bufs=2) as outp,
            tc.tile_pool(name="lptp", bufs=1) as lptp,
            tc.tile_pool(name="pz", bufs=2, space="PSUM") as pzp,
            tc.tile_pool(name="plp", bufs=2, space="PSUM") as plpp,
            tc.tile_pool(name="po", bufs=2, space="PSUM") as pop,
        ):
            smat_sb = constp.tile([128, 4, 128], F32R)
            w2_sb = constp.tile([128, n_trees, 2, CLASSES], F16)

            GROUP = 5
            first_mm = [None]

            # chunk-0 x load first: it heads the critical path
            xt_tiles = [None] * n_chunks

            def load_x(ci, split=False):
                xa = sb.tile([128, K8, CHUNK], F8, tag="xt8")
                xb = sb.tile([128, K16, CHUNK], F16, tag="xt16")
                if not split:
                    nc.sync.dma_start(xa[:, :, :], xt8[:, ci, :, :])
                    nc.sync.dma_start(xb[:, :, :], xt16[:, ci, :, :])
                xt_tiles[ci] = (xa, xb)
                return xa, xb

            wd_tiles = [None] * n_trees

            def load_wd(t, split=False):
                wa = constp.tile([128, K8, ND_PAD], F8, tag=f"wd8_{t}")
                wb = constp.tile([128, K16, ND_PAD], F16, tag=f"wd16_{t}")
                dmas = []
                if not split:
                    dmas.append(nc.sync.dma_start(wa[:, :, :], wd8[t, :, :, :]))
                    dmas.append(nc.sync.dma_start(wb[:, :, :], wd16[t, :, :, :]))
                wd_tiles[t] = (wa, wb)
                return dmas

            # startup: interleave split x/wd pieces so the first DR matmul
            # (reading k-tiles 0:2) waits on the minimum number of bytes
            xa0, xb0 = load_x(0, split=True)
            load_wd(0, split=True)
            wa0, wb0 = wd_tiles[0]
            nc.sync.dma_start(xa0[:, 0:4, :], xt8[:, 0, 0:4, :])
            nc.sync.dma_start(wa0[:, 0:4, :], wd8[0, :, 0:4, :])
            nc.sync.dma_start(xa0[:, 4:K8, :], xt8[:, 0, 4:K8, :])
            nc.sync.dma_start(wa0[:, 4:K8, :], wd8[0, :, 4:K8, :])
            nc.sync.dma_start(xb0[:, :, :], xt16[:, 0, :, :])
            nc.sync.dma_start(wb0[:, :, :], wd16[0, :, :, :])

            def emit_deferred():
                # behind the first matmul so they can't crowd the startup queues
                dmas = [nc.sync.dma_start(smat_sb[:, :, :], smat[:, :, :])]
                for t in range(1, n_trees):
                    dmas.extend(load_wd(t))

                for t in range(n_trees):
                    dmas.append(
                        nc.sync.dma_start(w2_sb[:, t, :, :], w2[t, :, :, :])
                    )
                for dma in dmas:
                    add_dep_helper(
                        dma.ins, first_mm[0].ins, sync=True,
                        reason="startup: critical pieces first",
                    )

            def emit_decision(t, xp, lpT):
                """mixed fp8-DoubleRow / f16 decision matmuls + Exp/z-copy."""
                xa, xb = xp
                wa, wb = wd_tiles[t]
                G = gp.tile([128, 4, CHUNK], F32R, tag="G")
                E = ep.tile([128, 2, CHUNK], F16, tag="E")
                last_exp = None
                fresh = first_mm[0] is None
                for dt_ in range(2):
                    psz = pzp.tile([128, CHUNK], F32, tag="psz")
                    for j in range(K8 // 2):
                        mm = nc.tensor.matmul(
                            psz[:, :],
                            wa[:, 2 * j : 2 * j + 2, dt_ * 128 : (dt_ + 1) * 128],
                            xa[:, 2 * j : 2 * j + 2, :],
                            start=(j == 0),
                            stop=False,
                            perf_mode=DR,
                        )
                        if first_mm[0] is None:
                            first_mm[0] = mm
                    for j in range(K16):
                        nc.tensor.matmul(
                            psz[:, :],
                            wb[:, j, dt_ * 128 : (dt_ + 1) * 128],
                            xb[:, j, :],
                            start=False,
                            stop=(j == K16 - 1),
                        )
                    # DVE copy is the ONLY psz reader (fast PSUM release);
                    # Exp reads the SBUF copy so the ACT queue's Ln-block +
                    # table-load latency can't block the next PE chain.
                    nc.vector.tensor_copy(G[:, 2 + dt_, :], psz[:, :])
                # one paired Exp over both halves (fewer ACT fixed overheads)
                last_exp = nc.scalar.activation(
                    E[:, :, :], G[:, 2:4, :], AF.Exp, scale=-Z_DESCALE
                )
                if fresh:
                    emit_deferred()
                return G, E, last_exp

            def emit_ln_block(group_G, group_E, last_exp):
                for G, E in zip(group_G, group_E):
                    ln = nc.scalar.activation(
                        G[:, 0:2, :], E[:, :, :], AF.Ln, bias=1.0
                    )
                    add_dep_helper(
                        ln.ins, last_exp.ins, sync=False,
                        reason="batch ACT Ln block after Exp block",
                    )

            def emit_smm(t0, group_G, lpT):
                # node-permuted S: leaf-half lt only needs node-tile lt, so
                # each half is a 2-deep accumulation (sp + z) instead of 4
                for i, G in enumerate(group_G):
                    t = t0 + i
                    plp = plpp.tile([128, 2, CHUNK], F32, tag="plp")
                    for lt in range(2):
                        nc.tensor.matmul(
                            plp[:, lt, :], smat_sb[:, lt, :], G[:, lt, :],
                            start=True, stop=False,
                        )
                        nc.tensor.matmul(
                            plp[:, lt, :], smat_sb[:, 2 + lt, :],
                            G[:, 2 + lt, :],
                            start=False, stop=True,
                        )
                    nc.scalar.activation(
                        lpT[:, t, 0:2, :], plp[:, :, :], AF.Exp, scale=-1.0
                    )

            def emit_mm2(ci, lpT, mid_cb=None):
                # mid_cb (the last group's S-matmuls) is woven into the middle
                # of the first accumulation chain: trees 0-4 give the in-order
                # PE ~4us of cover for the Ln-block + table-load latency that
                # gates those S-matmuls.
                c0 = ci * CHUNK
                n_acc = n_trees * 2
                for s in range(CHUNK // 128):
                    osb = outp.tile([128, CLASSES], F32, tag="osb")
                    for half in range(2):
                        po = pop.tile([128, 512], F32, tag="po")
                        i = 0
                        for t in range(n_trees):
                            if t == GROUP and mid_cb is not None:
                                mid_cb()
                                mid_cb = None
                            for lt in range(2):
                                nc.tensor.matmul(
                                    po[:, 0:500],
                                    lpT[:, t, lt, s * 128 : (s + 1) * 128],
                                    w2_sb[
                                        :, t, lt, half * 500 : half * 500 + 500
                                    ],
                                    start=(i == 0), stop=(i == n_acc - 1),
                                    skip_group_check=True,
                                )
                                i += 1
                        nc.vector.tensor_copy(
                            osb[:, half * 500 : half * 500 + 500], po[:, 0:500]
                        )
                    nc.sync.dma_start(
                        out[c0 + s * 128 : c0 + (s + 1) * 128, :], osb[:, :]
                    )

            for ci in range(n_chunks):
                xp = xt_tiles[ci]
                lpT = lptp.tile([128, n_trees, 2, CHUNK], F16, tag="lpT")
                # software pipeline: the S-block of group g is emitted after
                # two trees of group g+1 (far enough for the Exp->Ln latency,
                # close enough that G-tile reuse (gp bufs=5) can't deadlock)
                pend = None  # (t0, group_G) awaiting S-matmuls
                for t0 in range(0, n_trees, GROUP):
                    group = list(range(t0, min(t0 + GROUP, n_trees)))
                    group_G, group_E, last_exp = [], [], None
                    for i, t in enumerate(group):
                        G, E, last_exp = emit_decision(t, xp, lpT)
                        group_G.append(G)
                        group_E.append(E)
                        if pend is not None and i == 1:
                            emit_smm(*pend, lpT)
                            pend = None
                    emit_ln_block(group_G, group_E, last_exp)
                    pend = (t0, group_G)
                # prefetch next chunk's x while S/emit_mm2 fill the PE
                if ci + 1 < n_chunks:
                    load_x(ci + 1)
                pend_t0, pend_G = pend
                emit_mm2(
                    ci, lpT, mid_cb=lambda: emit_smm(pend_t0, pend_G, lpT)
                )
    nc.compile()
    return nc


def _node_perm():
    # node-tile lt holds exactly the path nodes of leaf-half lt: the shared
    # root (node 0) is duplicated into the otherwise-unused 256th slot.
    p0 = [0, 1] + [2**n - 1 + j for n in range(2, 8) for j in range(2 ** (n - 1))]
    p1 = [0, 2] + [
        2**n - 1 + j for n in range(2, 8) for j in range(2 ** (n - 1), 2**n)
    ]
    return [p0, p1]


def _smat_np():
    # k=lt: path indicator (multiplies softplus(-z)); k=2+lt: branch bits
    # scaled by 1/1024 (multiplies the raw psum = 1024*z).
    perm = _node_perm()
    S = np.zeros((128, 4, 128), np.float32)
    for lt in range(2):
        inv = {node: p for p, node in enumerate(perm[lt])}
        for ql in range(128):
            q = lt * 128 + ql
            for n in range(8):
                node = 2**n - 1 + (q >> (8 - n))
                b = (q >> (7 - n)) & 1
                S[inv[node], lt, ql] += 1.0
                S[inv[node], 2 + lt, ql] += b * np.float32(Z_DESCALE)
    return S


def _prep_weights(w_d, w_l, n_trees=N_TREES):
    fp8 = ml_dtypes.float8_e4m3
    w_l = np.asarray(w_l, dtype=np.float32)
    m = w_l.max(axis=-1, keepdims=True)
    e = np.exp(w_l - m, dtype=np.float32)
    sm = e / e.sum(axis=-1, keepdims=True)
    w2 = (sm[:, 0::2, :] + sm[:, 1::2, :]) * np.float32(1.0 / n_trees)
    # [t, 256, C] -> pre-tiled [t, 128, 2, C]
    w2 = np.ascontiguousarray(
        w2.reshape(n_trees, 2, 128, CLASSES).transpose(0, 2, 1, 3)
    ).astype(np.float16)
    wd_p = np.zeros((n_trees, IN_DIM, ND_PAD), np.float32)
    wd_p[:, :, : w_d.shape[2]] = w_d * np.float32(WD_SCALE)
    # permute decision-node columns so node-tile lt serves leaf-half lt
    perm = np.array(_node_perm()).reshape(-1)
    wd_p = wd_p[:, :, perm]
    # [t, (k p), d] -> pre-tiled [t, 128, k, d], split fp8/f16 k ranges
    wd_t = wd_p.reshape(n_trees, KI, 128, ND_PAD).transpose(0, 2, 1, 3)
    wd_8 = np.ascontiguousarray(wd_t[:, :, :K8]).astype(fp8)
    wd_16 = np.ascontiguousarray(wd_t[:, :, K8:]).astype(np.float16)
    return wd_8, wd_16, _smat_np(), w2


last_bass_results = None


def kernel(x, w_d, w_l):
    global last_bass_results
    x = np.asarray(x)
    wd_8, wd_16, S, w2 = _prep_weights(np.asarray(w_d), np.asarray(w_l))
    xs = x * np.float32(X_SCALE)
    in_maps = []
    for c in range(N_CORES):
        # [b_loc, IN_DIM] -> [128, n_chunks, k, CHUNK]:
        # xt[p, ci, k, n] = xs[c*B_LOC + ci*CHUNK + n, k*128 + p]
        xc = xs[c * B_LOC : (c + 1) * B_LOC, :]
        xct = xc.reshape(N_CHUNKS, CHUNK, KI, 128).transpose(3, 0, 2, 1)
        x_8 = np.ascontiguousarray(xct[:, :, :K8]).astype(ml_dtypes.float8_e4m3)
        x_16 = np.ascontiguousarray(xct[:, :, K8:]).astype(np.float16)
        in_maps.append(
            {"xt8": x_8, "xt16": x_16, "wd8": wd_8, "wd16": wd_16,
             "smat": S, "w2": w2}
        )
    if "nc" not in _CACHE:
        _CACHE["nc"] = _build()
    res = run_bass_kernel_spmd(_CACHE["nc"], in_maps, core_ids=list(range(N_CORES)))
    last_bass_results = res
    return np.concatenate([res.results[c]["out"] for c in range(N_CORES)], axis=0)


# revision 15
# speedup vs baseline: 1.2488x; 1.0349x over previous
"""Trainium2 Bass kernel for nn_DiffForest (soft decision forest forward).

Math: per tree t, z = x @ w_d[t]; p = sigmoid(z); leaf path probs are products
of 8 factors p/(1-p) down a depth-8 tree; output = sum_t leaf_prob @ softmax(w_l[t]) / 10.

Kernel formulation (all on device except small weight prep):
  - The 512 "leaves" come in identical pairs -> fold to 256 paths; fold the
    pair-sum + 1/n_trees into the leaf weight matrix w2 (host, exact).
  - Path products move to log space:  -log P[q] = sum_path softplus(-z) + sum_{branch=1} z
    which is a matmul with a constant matrix S [512, 256]:
        A = S^T @ [softplus(-z); z],   leaf_prob^T = exp(-A)   ([256 paths, batch])
    softplus(-z) = ln(1 + exp(-z)) via the Exp/Ln activation tables.
  - Decision matmul: mixed precision. 14 of 16 k-tiles run fp8e4 (x scaled by
    16, w_d by 64) with MatmulPerfMode.DoubleRow (two k-tiles per pass, 2x PE
    throughput); the last 2 k-tiles run f16 at the same 1024 product scale and
    accumulate into the same PSUM. Full-fp8 measures 1.95e-2 rel err vs the
    2e-2 gate; the f16 fraction buys the error margin back.
    The 1/1024 descale folds into the Exp activation scale and the z-half of S.
  - S-matmul in fp32r; leaf matmul in f16 (fp8 there costs ~1.2e-2 rel err).
  - All dram tensors are host-pre-tiled so every DMA moves long contiguous
    lines per partition (256B lines measured ~200GB/s; 4-8KB lines fix that).
  - The S-matmul block of group g is emitted after two trees of group g+1 so
    the in-order PE rides through the group's Exp->Ln->table-load latency.
  - Sharding: data-parallel over batch; each of the 8 cores takes 2048 rows of x,
    weights replicated, no collectives.
"""

import numpy as np
import ml_dtypes

import concourse.bacc as bacc
import concourse.mybir as mybir
import concourse.tile as tile
from concourse.tile import add_dep_helper
from concourse.bass_utils import run_bass_kernel_spmd

N_CORES = 8
BATCH = 16384
B_LOC = BATCH // N_CORES        # 2048 rows per core
IN_DIM = 2048
N_TREES = 10
ND_PAD = 256                    # decision nodes padded 255 -> 256
NQ = 256                        # folded path (leaf) count
CLASSES = 1000
CHUNK = 512                     # batch columns processed per chunk
KI = IN_DIM // 128              # 16 contraction tiles for the decision matmul
K8 = 14                         # k-tiles in fp8 (DoubleRow pairs)
K16 = KI - K8                   # k-tiles in f16
N_CHUNKS = B_LOC // CHUNK

BF16 = mybir.dt.bfloat16
F32 = mybir.dt.float32
F32R = mybir.dt.float32r
F16 = mybir.dt.float16
F8 = mybir.dt.float8e4
AF = mybir.ActivationFunctionType
DR = mybir.MatmulPerfMode.DoubleRow

X_SCALE = 16.0                  # x -> fp8/f16
WD_SCALE = 64.0                 # w_d -> fp8/f16
Z_DESCALE = 1.0 / (X_SCALE * WD_SCALE)   # psum holds 1024*z

_CACHE = {}


def _build(b_loc=B_LOC, n_trees=N_TREES):
    n_chunks = b_loc // CHUNK
    nc = bacc.Bacc("TRN2", target_bir_lowering=False)
    # host-pre-tiled layouts: partition dim first, contiguous k*free lines
    xt8 = nc.dram_tensor("xt8", (128, n_chunks, K8, CHUNK), F8, kind="ExternalInput")
    xt16 = nc.dram_tensor(
        "xt16", (128, n_chunks, K16, CHUNK), F16, kind="ExternalInput"
    )
    wd8 = nc.dram_tensor("wd8", (n_trees, 128, K8, ND_PAD), F8, kind="ExternalInput")
    wd16 = nc.dram_tensor(
        "wd16", (n_trees, 128, K16, ND_PAD), F16, kind="ExternalInput"
    )
    smat = nc.dram_tensor("smat", (128, 4, 128), F32R, kind="ExternalInput")
    w2 = nc.dram_tensor("w2", (n_trees, 128, 2, CLASSES), F16, kind="ExternalInput")
    out = nc.dram_tensor("out", (b_loc, CLASSES), F32, kind="ExternalOutput")

    with tile.TileContext(nc) as tc:
        with (
            tc.tile_pool(name="const", bufs=1) as constp,
            tc.tile_pool(name="sb", bufs=2) as sb,
            tc.tile_pool(name="ep", bufs=5) as ep,
            tc.tile_pool(name="gp", bufs=6) as gp,
            tc.tile_pool(name="outp", bufs=2) as outp,
            tc.tile_pool(name="lptp", bufs=1) as lptp,
            tc.tile_pool(name="pz", bufs=2, space="PSUM") as pzp,
            tc.tile_pool(name="plp", bufs=2, space="PSUM") as plpp,
            tc.tile_pool(name="po", bufs=2, space="PSUM") as pop,
        ):
            smat_sb = constp.tile([128, 4, 128], F32R)
            w2_sb = constp.tile([128, n_trees, 2, CLASSES], F16)

            # preload the ACT table set that holds BOTH exp and ln: the
            # insert_act_table_loads fixpoint then never needs a swap (17
            # swaps x 1283ns otherwise, each also stretching the Ln->Exp
            # critical chain at group boundaries)
            try:
                from concourse.hw_specs import get_activation_tables

                _sets = list(get_activation_tables(nc.m.arch).values())
                _set_id = next(
                    i for i, s in enumerate(_sets)
                    if AF.Exp in s and AF.Ln in s
                )
            except Exception:
                _set_id = 6
            nc.scalar.add_instruction(
                mybir.InstLoadActFuncSet(
                    name=nc.get_next_instruction_name(),
                    act_func_set_id=_set_id,
                    ins=[],
                    outs=[],
                )
            )

            GROUP = 5
            first_mm = [None]

            # chunk-0 x load first: it heads the critical path
            xt_tiles = [None] * n_chunks

            def load_x(ci, split=False):
                xa = sb.tile([128, K8, CHUNK], F8, tag="xt8")
                xb = sb.tile([128, K16, CHUNK], F16, tag="xt16")
                if not split:
                    nc.sync.dma_start(xa[:, :, :], xt8[:, ci, :, :])
                    nc.sync.dma_start(xb[:, :, :], xt16[:, ci, :, :])
                xt_tiles[ci] = (xa, xb)
                return xa, xb

            wd_tiles = [None] * n_trees

            def load_wd(t, split=False):
                wa = constp.tile([128, K8, ND_PAD], F8, tag=f"wd8_{t}")
                wb = constp.tile([128, K16, ND_PAD], F16, tag=f"wd16_{t}")
                dmas = []
                if not split:
                    dmas.append(nc.sync.dma_start(wa[:, :, :], wd8[t, :, :, :]))
                    dmas.append(nc.sync.dma_start(wb[:, :, :], wd16[t, :, :, :]))
                wd_tiles[t] = (wa, wb)
                return dmas

            # startup: interleave split x/wd pieces so the first DR matmul
            # (reading k-tiles 0:2) waits on the minimum number of bytes
            xa0, xb0 = load_x(0, split=True)
            load_wd(0, split=True)
            wa0, wb0 = wd_tiles[0]
            h = K8 // 2
            nc.sync.dma_start(xa0[:, 0:h, :], xt8[:, 0, 0:h, :])
            nc.sync.dma_start(wa0[:, 0:h, :], wd8[0, :, 0:h, :])
            nc.sync.dma_start(xa0[:, h:K8, :], xt8[:, 0, h:K8, :])
            nc.sync.dma_start(wa0[:, h:K8, :], wd8[0, :, h:K8, :])
            nc.sync.dma_start(xb0[:, :, :], xt16[:, 0, :, :])
            nc.sync.dma_start(wb0[:, :, :], wd16[0, :, :, :])

            def emit_deferred():
                # behind the first matmul so they can't crowd the startup queues
                dmas = [nc.sync.dma_start(smat_sb[:, :, :], smat[:, :, :])]
                for t in range(1, n_trees):
                    dmas.extend(load_wd(t))

                for t in range(n_trees):
                    dmas.append(
                        nc.sync.dma_start(w2_sb[:, t, :, :], w2[t, :, :, :])
                    )
                for dma in dmas:
                    add_dep_helper(
                        dma.ins, first_mm[0].ins, sync=True,
                        reason="startup: critical pieces first",
                    )

            def emit_decision(t, xp, lpT):
                """mixed fp8-DoubleRow / f16 decision matmuls + Exp/z-copy."""
                xa, xb = xp
                wa, wb = wd_tiles[t]
                G = gp.tile([128, 4, CHUNK], F32R, tag="G")
                E = ep.tile([128, 2, CHUNK], F16, tag="E")
                last_exp = None
                fresh = first_mm[0] is None
                for dt_ in range(2):
                    psz = pzp.tile([128, CHUNK], F32, tag="psz")
                    for j in range(K8 // 2):
                        mm = nc.tensor.matmul(
                            psz[:, :],
                            wa[:, 2 * j : 2 * j + 2, dt_ * 128 : (dt_ + 1) * 128],
                            xa[:, 2 * j : 2 * j + 2, :],
                            start=(j == 0),
                            stop=False,
                            perf_mode=DR,
                        )
                        if first_mm[0] is None:
                            first_mm[0] = mm
                    for j in range(K16):
                        nc.tensor.matmul(
                            psz[:, :],
                            wb[:, j, dt_ * 128 : (dt_ + 1) * 128],
                            xb[:, j, :],
                            start=False,
                            stop=(j == K16 - 1),
                        )
                    # DVE copy is the ONLY psz reader (fast PSUM release);
                    # Exp reads the SBUF copy so the ACT queue's Ln-block +
                    # table-load latency can't block the next PE chain.
                    nc.vector.tensor_copy(G[:, 2 + dt_, :], psz[:, :])
                # one paired Exp over both halves (fewer ACT fixed overheads)
                last_exp = nc.scalar.activation(
                    E[:, :, :], G[:, 2:4, :], AF.Exp, scale=-Z_DESCALE
                )
                if fresh:
                    emit_deferred()
                return G, E, last_exp

            def emit_ln_block(group_G, group_E, last_exp):
                for G, E in zip(group_G, group_E):
                    ln = nc.scalar.activation(
                        G[:, 0:2, :], E[:, :, :], AF.Ln, bias=1.0
                    )
                    add_dep_helper(
                        ln.ins, last_exp.ins, sync=False,
                        reason="batch ACT Ln block after Exp block",
                    )

            def emit_smm(t0, group_G, lpT):
                # node-permuted S: leaf-half lt only needs node-tile lt, so
                # each half is a 2-deep accumulation (sp + z) instead of 4
                for i, G in enumerate(group_G):
                    t = t0 + i
                    plp = plpp.tile([128, 2, CHUNK], F32, tag="plp")
                    for lt in range(2):
                        nc.tensor.matmul(
                            plp[:, lt, :], smat_sb[:, lt, :], G[:, lt, :],
                            start=True, stop=False,
                        )
                        nc.tensor.matmul(
                            plp[:, lt, :], smat_sb[:, 2 + lt, :],
                            G[:, 2 + lt, :],
                            start=False, stop=True,
                        )
                    nc.scalar.activation(
                        lpT[:, t, 0:2, :], plp[:, :, :], AF.Exp, scale=-1.0
                    )

            def emit_mm2(ci, lpT, mid_cb=None):
                # mid_cb (the last group's S-matmuls) is woven into the middle
                # of the first accumulation chain: trees 0-4 give the in-order
                # PE ~4us of cover for the Ln-block + table-load latency that
                # gates those S-matmuls.
                c0 = ci * CHUNK
                n_acc = n_trees * 2
                for s in range(CHUNK // 128):
                    osb = outp.tile([128, CLASSES], F32, tag="osb")
                    for half in range(2):
                        po = pop.tile([128, 512], F32, tag="po")
                        i = 0
                        for t in range(n_trees):
                            if t == GROUP and mid_cb is not None:
                                mid_cb()
                                mid_cb = None
                            for lt in range(2):
                                nc.tensor.matmul(
                                    po[:, 0:500],
                                    lpT[:, t, lt, s * 128 : (s + 1) * 128],
                                    w2_sb[
                                        :, t, lt, half * 500 : half * 500 + 500
                                    ],
                                    start=(i == 0), stop=(i == n_acc - 1),
                                    skip_group_check=True,
                                )
                                i += 1
                        nc.vector.tensor_copy(
                            osb[:, half * 500 : half * 500 + 500], po[:, 0:500]
                        )
                    nc.sync.dma_start(
                        out[c0 + s * 128 : c0 + (s + 1) * 128, :], osb[:, :]
                    )

            for ci in range(n_chunks):
                xp = xt_tiles[ci]
                lpT = lptp.tile([128, n_trees, 2, CHUNK], F16, tag="lpT")
                # software pipeline: the S-block of group g is emitted after
                # two trees of group g+1 (far enough for the Exp->Ln latency,
                # close enough that G-tile reuse (gp bufs=5) can't deadlock)
                pend = None  # (t0, group_G) awaiting S-matmuls
                for t0 in range(0, n_trees, GROUP):
                    group = list(range(t0, min(t0 + GROUP, n_trees)))
                    group_G, group_E, last_exp = [], [], None
                    for i, t in enumerate(group):
                        G, E, last_exp = emit_decision(t, xp, lpT)
                        group_G.append(G)
                        group_E.append(E)
                        if pend is not None and i == 1:
                            emit_smm(*pend, lpT)
                            pend = None
                    emit_ln_block(group_G, group_E, last_exp)
                    pend = (t0, group_G)
                # prefetch next chunk's x while S/emit_mm2 fill the PE
                if ci + 1 < n_chunks:
                    load_x(ci + 1)
                pend_t0, pend_G = pend
                emit_mm2(
                    ci, lpT, mid_cb=lambda: emit_smm(pend_t0, pend_G, lpT)
                )
    nc.compile()
    return nc


def _node_perm():
    # node-tile lt holds exactly the path nodes of leaf-half lt: the shared
    # root (node 0) is duplicated into the otherwise-unused 256th slot.
    p0 = [0, 1] + [2**n - 1 + j for n in range(2, 8) for j in range(2 ** (n - 1))]
    p1 = [0, 2] + [
        2**n - 1 + j for n in range(2, 8) for j in range(2 ** (n - 1), 2**n)
    ]
    return [p0, p1]


def _smat_np():
    # k=lt: path indicator (multiplies softplus(-z)); k=2+lt: branch bits
    # scaled by 1/1024 (multiplies the raw psum = 1024*z).
    perm = _node_perm()
    S = np.zeros((128, 4, 128), np.float32)
    for lt in range(2):
        inv = {node: p for p, node in enumerate(perm[lt])}
        for ql in range(128):
            q = lt * 128 + ql
            for n in range(8):
                node = 2**n - 1 + (q >> (8 - n))
                b = (q >> (7 - n)) & 1
                S[inv[node], lt, ql] += 1.0
                S[inv[node], 2 + lt, ql] += b * np.float32(Z_DESCALE)
    return S


def _prep_weights(w_d, w_l, n_trees=N_TREES):
    fp8 = ml_dtypes.float8_e4m3
    w_l = np.asarray(w_l, dtype=np.float32)
    m = w_l.max(axis=-1, keepdims=True)
    e = np.exp(w_l - m, dtype=np.float32)
    sm = e / e.sum(axis=-1, keepdims=True)
    w2 = (sm[:, 0::2, :] + sm[:, 1::2, :]) * np.float32(1.0 / n_trees)
    # [t, 256, C] -> pre-tiled [t, 128, 2, C]
    w2 = np.ascontiguousarray(
        w2.reshape(n_trees, 2, 128, CLASSES).transpose(0, 2, 1, 3)
    ).astype(np.float16)
    wd_p = np.zeros((n_trees, IN_DIM, ND_PAD), np.float32)
    wd_p[:, :, : w_d.shape[2]] = w_d * np.float32(WD_SCALE)
    # permute decision-node columns so node-tile lt serves leaf-half lt
    perm = np.array(_node_perm()).reshape(-1)
    wd_p = wd_p[:, :, perm]
    # [t, (k p), d] -> pre-tiled [t, 128, k, d], split fp8/f16 k ranges
    wd_t = wd_p.reshape(n_trees, KI, 128, ND_PAD).transpose(0, 2, 1, 3)
    wd_8 = np.ascontiguousarray(wd_t[:, :, :K8]).astype(fp8)
    wd_16 = np.ascontiguousarray(wd_t[:, :, K8:]).astype(np.float16)
    return wd_8, wd_16, _smat_np(), w2


last_bass_results = None


def kernel(x, w_d, w_l):
    global last_bass_results
    x = np.asarray(x)
    wd_8, wd_16, S, w2 = _prep_weights(np.asarray(w_d), np.asarray(w_l))
    xs = x * np.float32(X_SCALE)
    in_maps = []
    for c in range(N_CORES):
        # [b_loc, IN_DIM] -> [128, n_chunks, k, CHUNK]:
        # xt[p, ci, k, n] = xs[c*B_LOC + ci*CHUNK + n, k*128 + p]
        xc = xs[c * B_LOC : (c + 1) * B_LOC, :]
        xct = xc.reshape(N_CHUNKS, CHUNK, KI, 128).transpose(3, 0, 2, 1)
        x_8 = np.ascontiguousarray(xct[:, :, :K8]).astype(ml_dtypes.float8_e4m3)
        x_16 = np.ascontiguousarray(xct[:, :, K8:]).astype(np.float16)
        in_maps.append(
            {"xt8": x_8, "xt16": x_16, "wd8": wd_8, "wd16": wd_16,
             "smat": S, "w2": w2}
        )
    if "nc" not in _CACHE:
        _CACHE["nc"] = _build()
    res = run_bass_kernel_spmd(_CACHE["nc"], in_maps, core_ids=list(range(N_CORES)))
    last_bass_results = res
    return np.concatenate([res.results[c]["out"] for c in range(N_CORES)], axis=0)


# revision 16
# speedup vs baseline: 1.2705x; 1.0174x over previous
"""Trainium2 Bass kernel for nn_DiffForest (soft decision forest forward).

Math: per tree t, z = x @ w_d[t]; p = sigmoid(z); leaf path probs are products
of 8 factors p/(1-p) down a depth-8 tree; output = sum_t leaf_prob @ softmax(w_l[t]) / 10.

Kernel formulation (all on device except small weight prep):
  - The 512 "leaves" come in identical pairs -> fold to 256 paths; fold the
    pair-sum + 1/n_trees into the leaf weight matrix w2 (host, exact).
  - Path products move to log space:  -log P[q] = sum_path softplus(-z) + sum_{branch=1} z
    which is a matmul with a constant matrix S [512, 256]:
        A = S^T @ [softplus(-z); z],   leaf_prob^T = exp(-A)   ([256 paths, batch])
    softplus(-z) = ln(1 + exp(-z)) via the Exp/Ln activation tables.
  - Decision matmul: mixed precision. 14 of 16 k-tiles run fp8e4 (x scaled by
    16, w_d by 64) with MatmulPerfMode.DoubleRow (two k-tiles per pass, 2x PE
    throughput); the last 2 k-tiles run f16 at the same 1024 product scale and
    accumulate into the same PSUM. Full-fp8 measures 1.95e-2 rel err vs the
    2e-2 gate; the f16 fraction buys the error margin back.
    The 1/1024 descale folds into the Exp activation scale and the z-half of S.
  - S-matmul in fp32r; leaf matmul in f16 (fp8 there costs ~1.2e-2 rel err).
  - All dram tensors are host-pre-tiled so every DMA moves long contiguous
    lines per partition (256B lines measured ~200GB/s; 4-8KB lines fix that).
  - The S-matmul block of group g is emitted after two trees of group g+1 so
    the in-order PE rides through the group's Exp->Ln->table-load latency.
  - Sharding: data-parallel over batch; each of the 8 cores takes 2048 rows of x,
    weights replicated, no collectives.
"""

import numpy as np
import ml_dtypes

import concourse.bacc as bacc
import concourse.mybir as mybir
import concourse.tile as tile
from concourse.tile import add_dep_helper
from concourse.bass_utils import run_bass_kernel_spmd

N_CORES = 8
BATCH = 16384
B_LOC = BATCH // N_CORES        # 2048 rows per core
IN_DIM = 2048
N_TREES = 10
ND_PAD = 256                    # decision nodes padded 255 -> 256
NQ = 256                        # folded path (leaf) count
CLASSES = 1000
CHUNK = 512                     # batch columns processed per chunk
KI = IN_DIM // 128              # 16 contraction tiles for the decision matmul
K8 = 14                         # k-tiles in fp8 (DoubleRow pairs)
K16 = KI - K8                   # k-tiles in f16
N_CHUNKS = B_LOC // CHUNK

BF16 = mybir.dt.bfloat16
F32 = mybir.dt.float32
F32R = mybir.dt.float32r
F16 = mybir.dt.float16
F8 = mybir.dt.float8e4
AF = mybir.ActivationFunctionType
DR = mybir.MatmulPerfMode.DoubleRow

X_SCALE = 16.0                  # x -> fp8/f16
WD_SCALE = 64.0                 # w_d -> fp8/f16
Z_DESCALE = 1.0 / (X_SCALE * WD_SCALE)   # psum holds 1024*z

_CACHE = {}


def _build(b_loc=B_LOC, n_trees=N_TREES):
    n_chunks = b_loc // CHUNK
    nc = bacc.Bacc("TRN2", target_bir_lowering=False)
    # host-pre-tiled layouts: partition dim first, contiguous k*free lines
    xt8 = nc.dram_tensor("xt8", (128, n_chunks, K8, CHUNK), F8, kind="ExternalInput")
    xt16 = nc.dram_tensor(
        "xt16", (128, n_chunks, K16, CHUNK), F16, kind="ExternalInput"
    )
    wd8 = nc.dram_tensor("wd8", (n_trees, 128, K8, ND_PAD), F8, kind="ExternalInput")
    wd16 = nc.dram_tensor(
        "wd16", (n_trees, 128, K16, ND_PAD), F16, kind="ExternalInput"
    )
    smat = nc.dram_tensor("smat", (128, 4, 128), F32R, kind="ExternalInput")
    w2 = nc.dram_tensor("w2", (n_trees, 128, 2, CLASSES), F16, kind="ExternalInput")
    out = nc.dram_tensor("out", (b_loc, CLASSES), F32, kind="ExternalOutput")

    with tile.TileContext(nc) as tc:
        with (
            tc.tile_pool(name="const", bufs=1) as constp,
            tc.tile_pool(name="sb", bufs=2) as sb,
            tc.tile_pool(name="ep", bufs=5) as ep,
            tc.tile_pool(name="gp", bufs=6) as gp,
            tc.tile_pool(name="outp", bufs=2) as outp,
            tc.tile_pool(name="lptp", bufs=1) as lptp,
            tc.tile_pool(name="pz", bufs=2, space="PSUM") as pzp,
            tc.tile_pool(name="plp", bufs=2, space="PSUM") as plpp,
            tc.tile_pool(name="po", bufs=2, space="PSUM") as pop,
        ):
            smat_sb = constp.tile([128, 4, 128], F32R)
            w2_sb = constp.tile([128, n_trees, 2, CLASSES], F16)

            # preload the ACT table set that holds BOTH exp and ln: the
            # insert_act_table_loads fixpoint then never needs a swap (17
            # swaps x 1283ns otherwise, each also stretching the Ln->Exp
            # critical chain at group boundaries)
            try:
                from concourse.hw_specs import get_activation_tables

                _sets = list(get_activation_tables(nc.m.arch).values())
                _set_id = next(
                    i for i, s in enumerate(_sets)
                    if AF.Exp in s and AF.Ln in s
                )
            except Exception:
                _set_id = 6
            nc.scalar.add_instruction(
                mybir.InstLoadActFuncSet(
                    name=nc.get_next_instruction_name(),
                    act_func_set_id=_set_id,
                    ins=[],
                    outs=[],
                )
            )

            GROUP = 5
            first_mm = [None]

            # chunk-0 x load first: it heads the critical path
            xt_tiles = [None] * n_chunks

            def load_x(ci, split=False):
                xa = sb.tile([128, K8, CHUNK], F8, tag="xt8")
                xb = sb.tile([128, K16, CHUNK], F16, tag="xt16")
                if not split:
                    nc.sync.dma_start(xa[:, :, :], xt8[:, ci, :, :])
                    nc.sync.dma_start(xb[:, :, :], xt16[:, ci, :, :])
                xt_tiles[ci] = (xa, xb)
                return xa, xb

            wd_tiles = [None] * n_trees

            def load_wd(t, split=False):
                wa = constp.tile([128, K8, ND_PAD], F8, tag=f"wd8_{t}")
                wb = constp.tile([128, K16, ND_PAD], F16, tag=f"wd16_{t}")
                dmas = []
                if not split:
                    dmas.append(nc.sync.dma_start(wa[:, :, :], wd8[t, :, :, :]))
                    dmas.append(nc.sync.dma_start(wb[:, :, :], wd16[t, :, :, :]))
                wd_tiles[t] = (wa, wb)
                return dmas

            # startup: interleave split x/wd pieces so the first DR matmul
            # (reading k-tiles 0:2) waits on the minimum number of bytes
            xa0, xb0 = load_x(0, split=True)
            load_wd(0, split=True)
            wa0, wb0 = wd_tiles[0]
            for lo, hi in ((0, 4), (4, 9), (9, K8)):
                nc.sync.dma_start(wa0[:, lo:hi, :], wd8[0, :, lo:hi, :])
                nc.sync.dma_start(xa0[:, lo:hi, :], xt8[:, 0, lo:hi, :])
            nc.sync.dma_start(xb0[:, :, :], xt16[:, 0, :, :])
            nc.sync.dma_start(wb0[:, :, :], wd16[0, :, :, :])

            def emit_deferred():
                # behind the first matmul so they can't crowd the startup queues
                dmas = [nc.sync.dma_start(smat_sb[:, :, :], smat[:, :, :])]
                for t in range(1, n_trees):
                    dmas.extend(load_wd(t))

                for t in range(n_trees):
                    dmas.append(
                        nc.sync.dma_start(w2_sb[:, t, :, :], w2[t, :, :, :])
                    )
                for dma in dmas:
                    add_dep_helper(
                        dma.ins, first_mm[0].ins, sync=True,
                        reason="startup: critical pieces first",
                    )

            def emit_decision(t, xp, lpT):
                """mixed fp8-DoubleRow / f16 decision matmuls + Exp/z-copy."""
                xa, xb = xp
                wa, wb = wd_tiles[t]
                G = gp.tile([128, 4, CHUNK], F32R, tag="G")
                E = ep.tile([128, 2, CHUNK], F16, tag="E")
                last_exp = None
                fresh = first_mm[0] is None
                for dt_ in range(2):
                    psz = pzp.tile([128, CHUNK], F32, tag="psz")
                    for j in range(K8 // 2):
                        mm = nc.tensor.matmul(
                            psz[:, :],
                            wa[:, 2 * j : 2 * j + 2, dt_ * 128 : (dt_ + 1) * 128],
                            xa[:, 2 * j : 2 * j + 2, :],
                            start=(j == 0),
                            stop=False,
                            perf_mode=DR,
                        )
                        if first_mm[0] is None:
                            first_mm[0] = mm
                    for j in range(K16):
                        nc.tensor.matmul(
                            psz[:, :],
                            wb[:, j, dt_ * 128 : (dt_ + 1) * 128],
                            xb[:, j, :],
                            start=False,
                            stop=(j == K16 - 1),
                        )
                    # DVE copy is the ONLY psz reader (fast PSUM release);
                    # Exp reads the SBUF copy so the ACT queue's Ln-block +
                    # table-load latency can't block the next PE chain.
                    nc.vector.tensor_copy(G[:, 2 + dt_, :], psz[:, :])
                # one paired Exp over both halves (fewer ACT fixed overheads);
                # with the exp+ln table preloaded there is no swap cost, so Ln
                # follows immediately -- the S-matmuls' inputs are ready one
                # tree later instead of one group later
                nc.scalar.activation(
                    E[:, :, :], G[:, 2:4, :], AF.Exp, scale=-Z_DESCALE
                )
                nc.scalar.activation(G[:, 0:2, :], E[:, :, :], AF.Ln, bias=1.0)
                if fresh:
                    emit_deferred()
                return G

            def emit_smm(t0, group_G, lpT):
                # node-permuted S: leaf-half lt only needs node-tile lt, so
                # each half is a 2-deep accumulation (sp + z) instead of 4
                for i, G in enumerate(group_G):
                    t = t0 + i
                    plp = plpp.tile([128, 2, CHUNK], F32, tag="plp")
                    for lt in range(2):
                        nc.tensor.matmul(
                            plp[:, lt, :], smat_sb[:, lt, :], G[:, lt, :],
                            start=True, stop=False,
                        )
                        nc.tensor.matmul(
                            plp[:, lt, :], smat_sb[:, 2 + lt, :],
                            G[:, 2 + lt, :],
                            start=False, stop=True,
                        )
                    nc.scalar.activation(
                        lpT[:, t, 0:2, :], plp[:, :, :], AF.Exp, scale=-1.0
                    )

            def emit_mm2(ci, lpT, mid_cb=None):
                # mid_cb (the last group's S-matmuls) is woven into the middle
                # of the first accumulation chain: trees 0-4 give the in-order
                # PE ~4us of cover for the Ln-block + table-load latency that
                # gates those S-matmuls.
                c0 = ci * CHUNK
                n_acc = n_trees * 2
                for s in range(CHUNK // 128):
                    osb = outp.tile([128, CLASSES], F32, tag="osb")
                    for half in range(2):
                        po = pop.tile([128, 512], F32, tag="po")
                        i = 0
                        for t in range(n_trees):
                            if t == GROUP and mid_cb is not None:
                                mid_cb()
                                mid_cb = None
                            for lt in range(2):
                                nc.tensor.matmul(
                                    po[:, 0:500],
                                    lpT[:, t, lt, s * 128 : (s + 1) * 128],
                                    w2_sb[
                                        :, t, lt, half * 500 : half * 500 + 500
                                    ],
                                    start=(i == 0), stop=(i == n_acc - 1),
                                    skip_group_check=True,
                                )
                                i += 1
                        nc.vector.tensor_copy(
                            osb[:, half * 500 : half * 500 + 500], po[:, 0:500]
                        )
                    nc.sync.dma_start(
                        out[c0 + s * 128 : c0 + (s + 1) * 128, :], osb[:, :]
                    )

            for ci in range(n_chunks):
                xp = xt_tiles[ci]
                lpT = lptp.tile([128, n_trees, 2, CHUNK], F16, tag="lpT")
                # software pipeline: the S-block of group g is emitted after
                # two trees of group g+1 (far enough for the Exp->Ln latency,
                # close enough that G-tile reuse (gp bufs=5) can't deadlock)
                pend = None  # (t0, group_G) awaiting S-matmuls
                for t0 in range(0, n_trees, GROUP):
                    group = list(range(t0, min(t0 + GROUP, n_trees)))
                    group_G = []
                    for i, t in enumerate(group):
                        group_G.append(emit_decision(t, xp, lpT))
                        if pend is not None and i == 1:
                            emit_smm(*pend, lpT)
                            pend = None
                    pend = (t0, group_G)
                # prefetch next chunk's x while S/emit_mm2 fill the PE
                if ci + 1 < n_chunks:
                    load_x(ci + 1)
                pend_t0, pend_G = pend
                emit_mm2(
                    ci, lpT, mid_cb=lambda: emit_smm(pend_t0, pend_G, lpT)
                )
    nc.compile()
    return nc


def _node_perm():
    # node-tile lt holds exactly the path nodes of leaf-half lt: the shared
    # root (node 0) is duplicated into the otherwise-unused 256th slot.
    p0 = [0, 1] + [2**n - 1 + j for n in range(2, 8) for j in range(2 ** (n - 1))]
    p1 = [0, 2] + [
        2**n - 1 + j for n in range(2, 8) for j in range(2 ** (n - 1), 2**n)
    ]
    return [p0, p1]


def _smat_np():
    # k=lt: path indicator (multiplies softplus(-z)); k=2+lt: branch bits
    # scaled by 1/1024 (multiplies the raw psum = 1024*z).
    perm = _node_perm()
    S = np.zeros((128, 4, 128), np.float32)
    for lt in range(2):
        inv = {node: p for p, node in enumerate(perm[lt])}
        for ql in range(128):
            q = lt * 128 + ql
            for n in range(8):
                node = 2**n - 1 + (q >> (8 - n))
                b = (q >> (7 - n)) & 1
                S[inv[node], lt, ql] += 1.0
                S[inv[node], 2 + lt, ql] += b * np.float32(Z_DESCALE)
    return S


def _prep_weights(w_d, w_l, n_trees=N_TREES):
    fp8 = ml_dtypes.float8_e4m3
    w_l = np.asarray(w_l, dtype=np.float32)
    m = w_l.max(axis=-1, keepdims=True)
    e = np.exp(w_l - m, dtype=np.float32)
    sm = e / e.sum(axis=-1, keepdims=True)
    w2 = (sm[:, 0::2, :] + sm[:, 1::2, :]) * np.float32(1.0 / n_trees)
    # [t, 256, C] -> pre-tiled [t, 128, 2, C]
    w2 = np.ascontiguousarray(
        w2.reshape(n_trees, 2, 128, CLASSES).transpose(0, 2, 1, 3)
    ).astype(np.float16)
    wd_p = np.zeros((n_trees, IN_DIM, ND_PAD), np.float32)
    wd_p[:, :, : w_d.shape[2]] = w_d * np.float32(WD_SCALE)
    # permute decision-node columns so node-tile lt serves leaf-half lt
    perm = np.array(_node_perm()).reshape(-1)
    wd_p = wd_p[:, :, perm]
    # [t, (k p), d] -> pre-tiled [t, 128, k, d], split fp8/f16 k ranges
    wd_t = wd_p.reshape(n_trees, KI, 128, ND_PAD).transpose(0, 2, 1, 3)
    wd_8 = np.ascontiguousarray(wd_t[:, :, :K8]).astype(fp8)
    wd_16 = np.ascontiguousarray(wd_t[:, :, K8:]).astype(np.float16)
    return wd_8, wd_16, _smat_np(), w2


last_bass_results = None


def kernel(x, w_d, w_l):
    global last_bass_results
    x = np.asarray(x)
    wd_8, wd_16, S, w2 = _prep_weights(np.asarray(w_d), np.asarray(w_l))
    xs = x * np.float32(X_SCALE)
    in_maps = []
    for c in range(N_CORES):
        # [b_loc, IN_DIM] -> [128, n_chunks, k, CHUNK]:
        # xt[p, ci, k, n] = xs[c*B_LOC + ci*CHUNK + n, k*128 + p]
        xc = xs[c * B_LOC : (c + 1) * B_LOC, :]
        xct = xc.reshape(N_CHUNKS, CHUNK, KI, 128).transpose(3, 0, 2, 1)
        x_8 = np.ascontiguousarray(xct[:, :, :K8]).astype(ml_dtypes.float8_e4m3)
        x_16 = np.ascontiguousarray(xct[:, :, K8:]).astype(np.float16)
        in_maps.append(
            {"xt8": x_8, "xt16": x_16, "wd8": wd_8, "wd16": wd_16,
             "smat": S, "w2": w2}
        )
    if "nc" not in _CACHE:
        _CACHE["nc"] = _build()
    res = run_bass_kernel_spmd(_CACHE["nc"], in_maps, core_ids=list(range(N_CORES)))
    last_bass_results = res
    return np.concatenate([res.results[c]["out"] for c in range(N_CORES)], axis=0)


# revision 17
# speedup vs baseline: 1.2712x; 1.0005x over previous
"""Trainium2 Bass kernel for nn_DiffForest (soft decision forest forward).

Math: per tree t, z = x @ w_d[t]; p = sigmoid(z); leaf path probs are products
of 8 factors p/(1-p) down a depth-8 tree; output = sum_t leaf_prob @ softmax(w_l[t]) / 10.

Kernel formulation (all on device except small weight prep):
  - The 512 "leaves" come in identical pairs -> fold to 256 paths; fold the
    pair-sum + 1/n_trees into the leaf weight matrix w2 (host, exact).
  - Path products move to log space:  -log P[q] = sum_path softplus(-z) + sum_{branch=1} z
    which is a matmul with a constant matrix S [512, 256]:
        A = S^T @ [softplus(-z); z],   leaf_prob^T = exp(-A)   ([256 paths, batch])
    softplus(-z) = ln(1 + exp(-z)) via the Exp/Ln activation tables.
  - Decision matmul: mixed precision. 14 of 16 k-tiles run fp8e4 (x scaled by
    16, w_d by 64) with MatmulPerfMode.DoubleRow (two k-tiles per pass, 2x PE
    throughput); the last 2 k-tiles run f16 at the same 1024 product scale and
    accumulate into the same PSUM. Full-fp8 measures 1.95e-2 rel err vs the
    2e-2 gate; the f16 fraction buys the error margin back.
    The 1/1024 descale folds into the Exp activation scale and the z-half of S.
  - S-matmul in fp32r; leaf matmul in f16 (fp8 there costs ~1.2e-2 rel err).
  - All dram tensors are host-pre-tiled so every DMA moves long contiguous
    lines per partition (256B lines measured ~200GB/s; 4-8KB lines fix that).
  - The S-matmul block of group g is emitted after two trees of group g+1 so
    the in-order PE rides through the group's Exp->Ln->table-load latency.
  - Sharding: data-parallel over batch; each of the 8 cores takes 2048 rows of x,
    weights replicated, no collectives.
"""

import numpy as np
import ml_dtypes

import concourse.bacc as bacc
import concourse.mybir as mybir
import concourse.tile as tile
from concourse.tile import add_dep_helper
from concourse.bass_utils import run_bass_kernel_spmd

N_CORES = 8
BATCH = 16384
B_LOC = BATCH // N_CORES        # 2048 rows per core
IN_DIM = 2048
N_TREES = 10
ND_PAD = 256                    # decision nodes padded 255 -> 256
NQ = 256                        # folded path (leaf) count
CLASSES = 1000
CHUNK = 512                     # batch columns processed per chunk
KI = IN_DIM // 128              # 16 contraction tiles for the decision matmul
K8 = 14                         # k-tiles in fp8 (DoubleRow pairs)
K16 = KI - K8                   # k-tiles in f16
N_CHUNKS = B_LOC // CHUNK

BF16 = mybir.dt.bfloat16
F32 = mybir.dt.float32
F32R = mybir.dt.float32r
F16 = mybir.dt.float16
F8 = mybir.dt.float8e4
AF = mybir.ActivationFunctionType
DR = mybir.MatmulPerfMode.DoubleRow

X_SCALE = 16.0                  # x -> fp8/f16
WD_SCALE = 64.0                 # w_d -> fp8/f16
Z_DESCALE = 1.0 / (X_SCALE * WD_SCALE)   # psum holds 1024*z

_CACHE = {}


def _build(b_loc=B_LOC, n_trees=N_TREES):
    n_chunks = b_loc // CHUNK
    nc = bacc.Bacc("TRN2", target_bir_lowering=False)
    # host-pre-tiled layouts: partition dim first, contiguous k*free lines
    xt8 = nc.dram_tensor("xt8", (128, n_chunks, K8, CHUNK), F8, kind="ExternalInput")
    xt16 = nc.dram_tensor(
        "xt16", (128, n_chunks, K16, CHUNK), F16, kind="ExternalInput"
    )
    wd8 = nc.dram_tensor("wd8", (n_trees, 128, K8, ND_PAD), F8, kind="ExternalInput")
    wd16 = nc.dram_tensor(
        "wd16", (n_trees, 128, K16, ND_PAD), F16, kind="ExternalInput"
    )
    smat = nc.dram_tensor("smat", (128, 4, 128), F32R, kind="ExternalInput")
    w2 = nc.dram_tensor("w2", (n_trees, 128, 2, CLASSES), F16, kind="ExternalInput")
    out = nc.dram_tensor("out", (b_loc, CLASSES), F32, kind="ExternalOutput")

    with tile.TileContext(nc) as tc:
        with (
            tc.tile_pool(name="const", bufs=1) as constp,
            tc.tile_pool(name="sb", bufs=2) as sb,
            tc.tile_pool(name="ep", bufs=5) as ep,
            tc.tile_pool(name="gp", bufs=6) as gp,
            tc.tile_pool(name="outp", bufs=2) as outp,
            tc.tile_pool(name="lptp", bufs=1) as lptp,
            tc.tile_pool(name="pz", bufs=2, space="PSUM") as pzp,
            tc.tile_pool(name="plp", bufs=2, space="PSUM") as plpp,
            tc.tile_pool(name="po", bufs=2, space="PSUM") as pop,
        ):
            smat_sb = constp.tile([128, 4, 128], F32R)
            w2_sb = constp.tile([128, n_trees, 2, CLASSES], F16)

            # preload the ACT table set that holds BOTH exp and ln: the
            # insert_act_table_loads fixpoint then never needs a swap (17
            # swaps x 1283ns otherwise, each also stretching the Ln->Exp
            # critical chain at group boundaries)
            try:
                from concourse.hw_specs import get_activation_tables

                _sets = list(get_activation_tables(nc.m.arch).values())
                _set_id = next(
                    i for i, s in enumerate(_sets)
                    if AF.Exp in s and AF.Ln in s
                )
            except Exception:
                _set_id = 6
            nc.scalar.add_instruction(
                mybir.InstLoadActFuncSet(
                    name=nc.get_next_instruction_name(),
                    act_func_set_id=_set_id,
                    ins=[],
                    outs=[],
                )
            )

            GROUP = 5
            first_mm = [None]

            # chunk-0 x load first: it heads the critical path
            xt_tiles = [None] * n_chunks

            def load_x(ci, split=False):
                xa = sb.tile([128, K8, CHUNK], F8, tag="xt8")
                xb = sb.tile([128, K16, CHUNK], F16, tag="xt16")
                if not split:
                    nc.sync.dma_start(xa[:, :, :], xt8[:, ci, :, :])
                    nc.sync.dma_start(xb[:, :, :], xt16[:, ci, :, :])
                xt_tiles[ci] = (xa, xb)
                return xa, xb

            wd_tiles = [None] * n_trees

            def load_wd(t, split=False):
                wa = constp.tile([128, K8, ND_PAD], F8, tag=f"wd8_{t}")
                wb = constp.tile([128, K16, ND_PAD], F16, tag=f"wd16_{t}")
                dmas = []
                if not split:
                    dmas.append(nc.sync.dma_start(wa[:, :, :], wd8[t, :, :, :]))
                    dmas.append(nc.sync.dma_start(wb[:, :, :], wd16[t, :, :, :]))
                wd_tiles[t] = (wa, wb)
                return dmas

            # startup: interleave split x/wd pieces so the first DR matmul
            # (reading k-tiles 0:2) waits on the minimum number of bytes
            xa0, xb0 = load_x(0, split=True)
            load_wd(0, split=True)
            wa0, wb0 = wd_tiles[0]
            h = K8 // 2
            for lo, hi in ((0, h), (h, K8)):
                nc.sync.dma_start(wa0[:, lo:hi, :], wd8[0, :, lo:hi, :])
                nc.sync.dma_start(xa0[:, lo:hi, :], xt8[:, 0, lo:hi, :])
            nc.sync.dma_start(xb0[:, :, :], xt16[:, 0, :, :])
            nc.sync.dma_start(wb0[:, :, :], wd16[0, :, :, :])

            def emit_deferred():
                # behind the first matmul so they can't crowd the startup queues
                dmas = [nc.sync.dma_start(smat_sb[:, :, :], smat[:, :, :])]
                for t in range(1, n_trees):
                    dmas.extend(load_wd(t))

                for t in range(n_trees):
                    dmas.append(
                        nc.sync.dma_start(w2_sb[:, t, :, :], w2[t, :, :, :])
                    )
                for dma in dmas:
                    add_dep_helper(
                        dma.ins, first_mm[0].ins, sync=True,
                        reason="startup: critical pieces first",
                    )

            def emit_decision(t, xp, lpT):
                """mixed fp8-DoubleRow / f16 decision matmuls + Exp/z-copy."""
                xa, xb = xp
                wa, wb = wd_tiles[t]
                G = gp.tile([128, 4, CHUNK], F32R, tag="G")
                E = ep.tile([128, 2, CHUNK], F16, tag="E")
                last_exp = None
                fresh = first_mm[0] is None
                for dt_ in range(2):
                    psz = pzp.tile([128, CHUNK], F32, tag="psz")
                    for j in range(K8 // 2):
                        mm = nc.tensor.matmul(
                            psz[:, :],
                            wa[:, 2 * j : 2 * j + 2, dt_ * 128 : (dt_ + 1) * 128],
                            xa[:, 2 * j : 2 * j + 2, :],
                            start=(j == 0),
                            stop=False,
                            perf_mode=DR,
                        )
                        if first_mm[0] is None:
                            first_mm[0] = mm
                    for j in range(K16):
                        nc.tensor.matmul(
                            psz[:, :],
                            wb[:, j, dt_ * 128 : (dt_ + 1) * 128],
                            xb[:, j, :],
                            start=False,
                            stop=(j == K16 - 1),
                        )
                    # DVE copy is the ONLY psz reader (fast PSUM release);
                    # Exp reads the SBUF copy so the ACT queue's Ln-block +
                    # table-load latency can't block the next PE chain.
                    nc.vector.tensor_copy(G[:, 2 + dt_, :], psz[:, :])
                # one paired Exp over both halves (fewer ACT fixed overheads);
                # with the exp+ln table preloaded there is no swap cost, so Ln
                # follows immediately -- the S-matmuls' inputs are ready one
                # tree later instead of one group later
                nc.scalar.activation(
                    E[:, :, :], G[:, 2:4, :], AF.Exp, scale=-Z_DESCALE
                )
                nc.scalar.activation(G[:, 0:2, :], E[:, :, :], AF.Ln, bias=1.0)
                if fresh:
                    emit_deferred()
                return G

            def emit_smm(t0, group_G, lpT):
                # node-permuted S: leaf-half lt only needs node-tile lt, so
                # each half is a 2-deep accumulation (sp + z) instead of 4
                for i, G in enumerate(group_G):
                    t = t0 + i
                    plp = plpp.tile([128, 2, CHUNK], F32, tag="plp")
                    for lt in range(2):
                        nc.tensor.matmul(
                            plp[:, lt, :], smat_sb[:, lt, :], G[:, lt, :],
                            start=True, stop=False,
                        )
                        nc.tensor.matmul(
                            plp[:, lt, :], smat_sb[:, 2 + lt, :],
                            G[:, 2 + lt, :],
                            start=False, stop=True,
                        )
                    nc.scalar.activation(
                        lpT[:, t, 0:2, :], plp[:, :, :], AF.Exp, scale=-1.0
                    )

            def emit_mm2(ci, lpT, mid_cb=None):
                # mid_cb (the last group's S-matmuls) is woven into the middle
                # of the first accumulation chain: trees 0-4 give the in-order
                # PE ~4us of cover for the Ln-block + table-load latency that
                # gates those S-matmuls.
                c0 = ci * CHUNK
                n_acc = n_trees * 2
                for s in range(CHUNK // 128):
                    osb = outp.tile([128, CLASSES], F32, tag="osb")
                    for half in range(2):
                        po = pop.tile([128, 512], F32, tag="po")
                        i = 0
                        for t in range(n_trees):
                            if t == GROUP and mid_cb is not None:
                                mid_cb()
                                mid_cb = None
                            for lt in range(2):
                                nc.tensor.matmul(
                                    po[:, 0:500],
                                    lpT[:, t, lt, s * 128 : (s + 1) * 128],
                                    w2_sb[
                                        :, t, lt, half * 500 : half * 500 + 500
                                    ],
                                    start=(i == 0), stop=(i == n_acc - 1),
                                    skip_group_check=True,
                                )
                                i += 1
                        nc.vector.tensor_copy(
                            osb[:, half * 500 : half * 500 + 500], po[:, 0:500]
                        )
                    nc.sync.dma_start(
                        out[c0 + s * 128 : c0 + (s + 1) * 128, :], osb[:, :]
                    )

            for ci in range(n_chunks):
                xp = xt_tiles[ci]
                lpT = lptp.tile([128, n_trees, 2, CHUNK], F16, tag="lpT")
                # software pipeline: the S-block of group g is emitted after
                # two trees of group g+1 (far enough for the Exp->Ln latency,
                # close enough that G-tile reuse (gp bufs=5) can't deadlock)
                pend = None  # (t0, group_G) awaiting S-matmuls
                for t0 in range(0, n_trees, GROUP):
                    group = list(range(t0, min(t0 + GROUP, n_trees)))
                    group_G = []
                    for i, t in enumerate(group):
                        group_G.append(emit_decision(t, xp, lpT))
                        if pend is not None and i == 1:
                            emit_smm(*pend, lpT)
                            pend = None
                    pend = (t0, group_G)
                # prefetch next chunk's x while S/emit_mm2 fill the PE
                if ci + 1 < n_chunks:
                    load_x(ci + 1)
                pend_t0, pend_G = pend
                emit_mm2(
                    ci, lpT, mid_cb=lambda: emit_smm(pend_t0, pend_G, lpT)
                )
    nc.compile()
    return nc


def _node_perm():
    # node-tile lt holds exactly the path nodes of leaf-half lt: the shared
    # root (node 0) is duplicated into the otherwise-unused 256th slot.
    p0 = [0, 1] + [2**n - 1 + j for n in range(2, 8) for j in range(2 ** (n - 1))]
    p1 = [0, 2] + [
        2**n - 1 + j for n in range(2, 8) for j in range(2 ** (n - 1), 2**n)
    ]
    return [p0, p1]


def _smat_np():
    # k=lt: path indicator (multiplies softplus(-z)); k=2+lt: branch bits
    # scaled by 1/1024 (multiplies the raw psum = 1024*z).
    perm = _node_perm()
    S = np.zeros((128, 4, 128), np.float32)
    for lt in range(2):
        inv = {node: p for p, node in enumerate(perm[lt])}
        for ql in range(128):
            q = lt * 128 + ql
            for n in range(8):
                node = 2**n - 1 + (q >> (8 - n))
                b = (q >> (7 - n)) & 1
                S[inv[node], lt, ql] += 1.0
                S[inv[node], 2 + lt, ql] += b * np.float32(Z_DESCALE)
    return S


def _prep_weights(w_d, w_l, n_trees=N_TREES):
    fp8 = ml_dtypes.float8_e4m3
    w_l = np.asarray(w_l, dtype=np.float32)
    m = w_l.max(axis=-1, keepdims=True)
    e = np.exp(w_l - m, dtype=np.float32)
    sm = e / e.sum(axis=-1, keepdims=True)
    w2 = (sm[:, 0::2, :] + sm[:, 1::2, :]) * np.float32(1.0 / n_trees)
    # [t, 256, C] -> pre-tiled [t, 128, 2, C]
    w2 = np.ascontiguousarray(
        w2.reshape(n_trees, 2, 128, CLASSES).transpose(0, 2, 1, 3)
    ).astype(np.float16)
    wd_p = np.zeros((n_trees, IN_DIM, ND_PAD), np.float32)
    wd_p[:, :, : w_d.shape[2]] = w_d * np.float32(WD_SCALE)
    # permute decision-node columns so node-tile lt serves leaf-half lt
    perm = np.array(_node_perm()).reshape(-1)
    wd_p = wd_p[:, :, perm]
    # [t, (k p), d] -> pre-tiled [t, 128, k, d], split fp8/f16 k ranges
    wd_t = wd_p.reshape(n_trees, KI, 128, ND_PAD).transpose(0, 2, 1, 3)
    wd_8 = np.ascontiguousarray(wd_t[:, :, :K8]).astype(fp8)
    wd_16 = np.ascontiguousarray(wd_t[:, :, K8:]).astype(np.float16)
    return wd_8, wd_16, _smat_np(), w2


last_bass_results = None


def kernel(x, w_d, w_l):
    global last_bass_results
    x = np.asarray(x)
    wd_8, wd_16, S, w2 = _prep_weights(np.asarray(w_d), np.asarray(w_l))
    xs = x * np.float32(X_SCALE)
    in_maps = []
    for c in range(N_CORES):
        # [b_loc, IN_DIM] -> [128, n_chunks, k, CHUNK]:
        # xt[p, ci, k, n] = xs[c*B_LOC + ci*CHUNK + n, k*128 + p]
        xc = xs[c * B_LOC : (c + 1) * B_LOC, :]
        xct = xc.reshape(N_CHUNKS, CHUNK, KI, 128).transpose(3, 0, 2, 1)
        x_8 = np.ascontiguousarray(xct[:, :, :K8]).astype(ml_dtypes.float8_e4m3)
        x_16 = np.ascontiguousarray(xct[:, :, K8:]).astype(np.float16)
        in_maps.append(
            {"xt8": x_8, "xt16": x_16, "wd8": wd_8, "wd16": wd_16,
             "smat": S, "w2": w2}
        )
    if "nc" not in _CACHE:
        _CACHE["nc"] = _build()
    res = run_bass_kernel_spmd(_CACHE["nc"], in_maps, core_ids=list(range(N_CORES)))
    last_bass_results = res
    return np.concatenate([res.results[c]["out"] for c in range(N_CORES)], axis=0)


# revision 20
# speedup vs baseline: 1.5071x; 1.1856x over previous
"""Trainium2 Bass kernel for nn_DiffForest (soft decision forest forward).

Math: per tree t, z = x @ w_d[t]; p = sigmoid(z); leaf path probs are products
of 8 factors p/(1-p) down a depth-8 tree; output = sum_t leaf_prob @ softmax(w_l[t]) / 10.

Kernel formulation (all on device except small weight prep):
  - The 512 "leaves" come in identical pairs -> fold to 256 paths; fold the
    pair-sum + 1/n_trees into the leaf weight matrix w2 (host, exact).
  - Path products move to log space:  -log P[q] = sum_path softplus(-z) + sum_{branch=1} z
    which is a matmul with a constant matrix S [512, 256]:
        A = S^T @ [softplus(-z); z],   leaf_prob^T = exp(-A)   ([256 paths, batch])
    softplus(-z) = ln(1 + exp(-z)) via the Exp/Ln activation tables.
  - Decision matmul: mixed precision. 14 of 16 k-tiles run fp8e4 (x scaled by
    16, w_d by 64) with MatmulPerfMode.DoubleRow (two k-tiles per pass, 2x PE
    throughput); the last 2 k-tiles run f16 at the same 1024 product scale and
    accumulate into the same PSUM. Full-fp8 measures 1.95e-2 rel err vs the
    2e-2 gate; the f16 fraction buys the error margin back.
    The 1/1024 descale folds into the Exp activation scale and the z-half of S.
  - S-matmul in fp32r; leaf matmul in f16 (fp8 there costs ~1.2e-2 rel err).
  - All dram tensors are host-pre-tiled so every DMA moves long contiguous
    lines per partition (256B lines measured ~200GB/s; 4-8KB lines fix that).
  - The S-matmul block of group g is emitted after two trees of group g+1 so
    the in-order PE rides through the group's Exp->Ln->table-load latency.
  - Sharding: data-parallel over batch; each of the 8 cores takes 2048 rows of x,
    weights replicated, no collectives.
"""

import numpy as np
import ml_dtypes

import concourse.bacc as bacc
import concourse.mybir as mybir
import concourse.tile as tile
from concourse.tile import add_dep_helper
from concourse.bass_utils import run_bass_kernel_spmd

N_CORES = 8
BATCH = 16384
B_LOC = BATCH // N_CORES        # 2048 rows per core
IN_DIM = 2048
N_TREES = 10
ND_PAD = 256                    # decision nodes padded 255 -> 256
NQ = 256                        # folded path (leaf) count
CLASSES = 1000
CHUNK = 512                     # batch columns processed per chunk
KI = IN_DIM // 128              # 16 contraction tiles for the decision matmul
K8 = 14                         # k-tiles in fp8 (DoubleRow pairs)
K16 = KI - K8                   # k-tiles in f16
N_CHUNKS = B_LOC // CHUNK

BF16 = mybir.dt.bfloat16
F32 = mybir.dt.float32
F32R = mybir.dt.float32r
F16 = mybir.dt.float16
F8 = mybir.dt.float8e4
AF = mybir.ActivationFunctionType
DR = mybir.MatmulPerfMode.DoubleRow

NF16 = 128                      # leaf classes kept f16 (largest w2 columns)
NC8 = CLASSES - NF16            # leaf classes in fp8 DoubleRow
C8H = NC8 // 2                  # fp8 class-chain width (psum bank limit)
W2_SCALE8 = 8192.0              # fp8 leaf weight scale
LP_SCALE = 64.0                 # lpT holds 64*exp(-A)
LN_LP_SCALE = float(np.log(LP_SCALE))

X_SCALE = 16.0                  # x -> fp8/f16
WD_SCALE = 64.0                 # w_d -> fp8/f16
Z_DESCALE = 1.0 / (X_SCALE * WD_SCALE)   # psum holds 1024*z

_CACHE = {}


def _build(b_loc=B_LOC, n_trees=N_TREES):
    n_chunks = b_loc // CHUNK
    nc = bacc.Bacc("TRN2", target_bir_lowering=False)
    # host-pre-tiled layouts: partition dim first, contiguous k*free lines
    xt8 = nc.dram_tensor("xt8", (128, n_chunks, K8, CHUNK), F8, kind="ExternalInput")
    xt16 = nc.dram_tensor(
        "xt16", (128, n_chunks, K16, CHUNK), F16, kind="ExternalInput"
    )
    wd8 = nc.dram_tensor("wd8", (n_trees, 128, K8, ND_PAD), F8, kind="ExternalInput")
    wd16 = nc.dram_tensor(
        "wd16", (n_trees, 128, K16, ND_PAD), F16, kind="ExternalInput"
    )
    smat = nc.dram_tensor("smat", (128, 4, 128), F32R, kind="ExternalInput")
    w2h = nc.dram_tensor("w2h", (n_trees, 128, 2, NF16), F16, kind="ExternalInput")
    w28 = nc.dram_tensor("w28", (n_trees, 128, 2, NC8), F8, kind="ExternalInput")
    out = nc.dram_tensor("out", (b_loc, CLASSES), F32, kind="ExternalOutput")

    with tile.TileContext(nc) as tc:
        with (
            tc.tile_pool(name="const", bufs=1) as constp,
            tc.tile_pool(name="sb", bufs=2) as sb,
            tc.tile_pool(name="ep", bufs=5) as ep,
            tc.tile_pool(name="gp", bufs=6) as gp,
            tc.tile_pool(name="outp", bufs=2) as outp,
            tc.tile_pool(name="lptp", bufs=1) as lptp,
            tc.tile_pool(name="lpt8p", bufs=1) as lpt8p,
            tc.tile_pool(name="pz", bufs=2, space="PSUM") as pzp,
            tc.tile_pool(name="plp", bufs=2, space="PSUM") as plpp,
            tc.tile_pool(name="po", bufs=2, space="PSUM") as pop,
        ):
            smat_sb = constp.tile([128, 4, 128], F32R)
            w2h_sb = constp.tile([128, n_trees, 2, NF16], F16)
            w28_sb = constp.tile([128, n_trees, 2, NC8], F8)

            # preload the ACT table set that holds BOTH exp and ln: the
            # insert_act_table_loads fixpoint then never needs a swap (17
            # swaps x 1283ns otherwise, each also stretching the Ln->Exp
            # critical chain at group boundaries)
            try:
                from concourse.hw_specs import get_activation_tables

                _sets = list(get_activation_tables(nc.m.arch).values())
                _set_id = next(
                    i for i, s in enumerate(_sets)
                    if AF.Exp in s and AF.Ln in s
                )
            except Exception:
                _set_id = 6
            nc.scalar.add_instruction(
                mybir.InstLoadActFuncSet(
                    name=nc.get_next_instruction_name(),
                    act_func_set_id=_set_id,
                    ins=[],
                    outs=[],
                )
            )

            GROUP = 5
            first_mm = [None]

            # chunk-0 x load first: it heads the critical path
            xt_tiles = [None] * n_chunks

            def load_x(ci, split=False):
                xa = sb.tile([128, K8, CHUNK], F8, tag="xt8")
                xb = sb.tile([128, K16, CHUNK], F16, tag="xt16")
                if not split:
                    nc.sync.dma_start(xa[:, :, :], xt8[:, ci, :, :])
                    nc.sync.dma_start(xb[:, :, :], xt16[:, ci, :, :])
                xt_tiles[ci] = (xa, xb)
                return xa, xb

            wd_tiles = [None] * n_trees

            def load_wd(t, split=False):
                wa = constp.tile([128, K8, ND_PAD], F8, tag=f"wd8_{t}")
                wb = constp.tile([128, K16, ND_PAD], F16, tag=f"wd16_{t}")
                dmas = []
                if not split:
                    dmas.append(nc.sync.dma_start(wa[:, :, :], wd8[t, :, :, :]))
                    dmas.append(nc.sync.dma_start(wb[:, :, :], wd16[t, :, :, :]))
                wd_tiles[t] = (wa, wb)
                return dmas

            # startup: interleave split x/wd pieces so the first DR matmul
            # (reading k-tiles 0:2) waits on the minimum number of bytes
            xa0, xb0 = load_x(0, split=True)
            load_wd(0, split=True)
            wa0, wb0 = wd_tiles[0]
            h = K8 // 2
            for lo, hi in ((0, h), (h, K8)):
                nc.sync.dma_start(wa0[:, lo:hi, :], wd8[0, :, lo:hi, :])
                nc.sync.dma_start(xa0[:, lo:hi, :], xt8[:, 0, lo:hi, :])
            nc.sync.dma_start(xb0[:, :, :], xt16[:, 0, :, :])
            nc.sync.dma_start(wb0[:, :, :], wd16[0, :, :, :])

            def emit_deferred():
                # behind the first matmul so they can't crowd the startup queues
                dmas = [nc.sync.dma_start(smat_sb[:, :, :], smat[:, :, :])]
                for t in range(1, n_trees):
                    dmas.extend(load_wd(t))

                for t in range(n_trees):
                    dmas.append(
                        nc.sync.dma_start(w2h_sb[:, t, :, :], w2h[t, :, :, :])
                    )
                    dmas.append(
                        nc.sync.dma_start(w28_sb[:, t, :, :], w28[t, :, :, :])
                    )
                for dma in dmas:
                    add_dep_helper(
                        dma.ins, first_mm[0].ins, sync=True,
                        reason="startup: critical pieces first",
                    )

            def emit_decision(t, xp, lpT):
                """mixed fp8-DoubleRow / f16 decision matmuls + Exp/z-copy."""
                xa, xb = xp
                wa, wb = wd_tiles[t]
                G = gp.tile([128, 4, CHUNK], F32R, tag="G")
                E = ep.tile([128, 2, CHUNK], F16, tag="E")
                last_exp = None
                fresh = first_mm[0] is None
                for dt_ in range(2):
                    psz = pzp.tile([128, CHUNK], F32, tag="psz")
                    for j in range(K8 // 2):
                        mm = nc.tensor.matmul(
                            psz[:, :],
                            wa[:, 2 * j : 2 * j + 2, dt_ * 128 : (dt_ + 1) * 128],
                            xa[:, 2 * j : 2 * j + 2, :],
                            start=(j == 0),
                            stop=False,
                            perf_mode=DR,
                        )
                        if first_mm[0] is None:
                            first_mm[0] = mm
                    for j in range(K16):
                        nc.tensor.matmul(
                            psz[:, :],
                            wb[:, j, dt_ * 128 : (dt_ + 1) * 128],
                            xb[:, j, :],
                            start=False,
                            stop=(j == K16 - 1),
                        )
                    # DVE copy is the ONLY psz reader (fast PSUM release);
                    # Exp reads the SBUF copy so the ACT queue's Ln-block +
                    # table-load latency can't block the next PE chain.
                    nc.vector.tensor_copy(G[:, 2 + dt_, :], psz[:, :])
                # one paired Exp over both halves (fewer ACT fixed overheads);
                # with the exp+ln table preloaded there is no swap cost, so Ln
                # follows immediately -- the S-matmuls' inputs are ready one
                # tree later instead of one group later
                nc.scalar.activation(
                    E[:, :, :], G[:, 2:4, :], AF.Exp, scale=-Z_DESCALE
                )
                nc.scalar.activation(G[:, 0:2, :], E[:, :, :], AF.Ln, bias=1.0)
                if fresh:
                    emit_deferred()
                return G

            def emit_smm(t0, group_G, lpT, lpT8):
                # node-permuted S: leaf-half lt only needs node-tile lt, so
                # each half is a 2-deep accumulation (sp + z) instead of 4.
                # lpT holds 64*exp(-A) in f16; a DVE cast makes the fp8 copy
                # for the small-magnitude class columns.
                for i, G in enumerate(group_G):
                    t = t0 + i
                    for lt in range(2):
                        plp = plpp.tile([128, CHUNK], F32, tag="plp")
                        nc.tensor.matmul(
                            plp[:, :], smat_sb[:, lt, :], G[:, lt, :],
                            start=True, stop=False,
                        )
                        nc.tensor.matmul(
                            plp[:, :], smat_sb[:, 2 + lt, :],
                            G[:, 2 + lt, :],
                            start=False, stop=True,
                        )
                        nc.scalar.activation(
                            lpT[:, t, lt, :], plp[:, :], AF.Exp, scale=-1.0
                        )
                    nc.vector.tensor_scalar_mul(
                        lpT8[:, t, 0:2, :], lpT[:, t, 0:2, :], LP_SCALE
                    )

            def emit_mm2(ci, lpT, lpT8, mid_cb=None):
                # per s-block: one f16 chain over the NF16 large-magnitude
                # class columns + two fp8-DoubleRow chains over the rest
                # (class order is permuted on the host, un-permuted after
                # gather).  mid_cb (the last group's S-matmuls) is woven into
                # the f16 chain: trees 0-4 give the in-order PE cover for the
                # ACT latency gating those S-matmuls.
                c0 = ci * CHUNK
                n_acc = n_trees * 2
                for s in range(CHUNK // 128):
                    osb = outp.tile([128, CLASSES], F32, tag="osb")
                    po = pop.tile([128, NF16], F32, tag="poA")
                    i = 0
                    for t in range(n_trees):
                        if t == GROUP and mid_cb is not None:
                            mid_cb()
                            mid_cb = None
                        for lt in range(2):
                            nc.tensor.matmul(
                                po[:, :],
                                lpT[:, t, lt, s * 128 : (s + 1) * 128],
                                w2h_sb[:, t, lt, :],
                                start=(i == 0), stop=(i == n_acc - 1),
                                skip_group_check=True,
                            )
                            i += 1
                    nc.vector.tensor_copy(osb[:, 0:NF16], po[:, :])
                    for half in range(2):
                        po8 = pop.tile([128, C8H], F32, tag="po8")
                        for t in range(n_trees):
                            nc.tensor.matmul(
                                po8[:, :],
                                lpT8[:, t, 0:2, s * 128 : (s + 1) * 128],
                                w28_sb[
                                    :, t, 0:2,
                                    half * C8H : half * C8H + C8H,
                                ],
                                start=(t == 0), stop=(t == n_trees - 1),
                                perf_mode=DR,
                            )
                        nc.vector.tensor_scalar_mul(
                            osb[:, NF16 + half * C8H : NF16 + half * C8H + C8H],
                            po8[:, :],
                            1.0 / (LP_SCALE * W2_SCALE8),
                        )
                    nc.sync.dma_start(
                        out[c0 + s * 128 : c0 + (s + 1) * 128, :], osb[:, :]
                    )

            for ci in range(n_chunks):
                xp = xt_tiles[ci]
                lpT = lptp.tile([128, n_trees, 2, CHUNK], F16, tag="lpT")
                lpT8 = lpt8p.tile([128, n_trees, 2, CHUNK], F8, tag="lpT8")
                # software pipeline: the S-block of group g is emitted after
                # two trees of group g+1 (far enough for the Exp->Ln latency,
                # close enough that G-tile reuse (gp bufs=5) can't deadlock)
                pend = None  # (t0, group_G) awaiting S-matmuls
                for t0 in range(0, n_trees, GROUP):
                    group = list(range(t0, min(t0 + GROUP, n_trees)))
                    group_G = []
                    for i, t in enumerate(group):
                        group_G.append(emit_decision(t, xp, lpT))
                        if pend is not None and i == 1:
                            emit_smm(*pend, lpT, lpT8)
                            pend = None
                    pend = (t0, group_G)
                # prefetch next chunk's x while S/emit_mm2 fill the PE
                if ci + 1 < n_chunks:
                    load_x(ci + 1)
                pend_t0, pend_G = pend
                emit_mm2(
                    ci, lpT, lpT8,
                    mid_cb=lambda: emit_smm(pend_t0, pend_G, lpT, lpT8),
                )
    nc.compile()
    return nc


def _node_perm():
    # node-tile lt holds exactly the path nodes of leaf-half lt: the shared
    # root (node 0) is duplicated into the otherwise-unused 256th slot.
    p0 = [0, 1] + [2**n - 1 + j for n in range(2, 8) for j in range(2 ** (n - 1))]
    p1 = [0, 2] + [
        2**n - 1 + j for n in range(2, 8) for j in range(2 ** (n - 1), 2**n)
    ]
    return [p0, p1]


def _smat_np():
    # k=lt: path indicator (multiplies softplus(-z)); k=2+lt: branch bits
    # scaled by 1/1024 (multiplies the raw psum = 1024*z).
    perm = _node_perm()
    S = np.zeros((128, 4, 128), np.float32)
    for lt in range(2):
        inv = {node: p for p, node in enumerate(perm[lt])}
        for ql in range(128):
            q = lt * 128 + ql
            for n in range(8):
                node = 2**n - 1 + (q >> (8 - n))
                b = (q >> (7 - n)) & 1
                S[inv[node], lt, ql] += 1.0
                S[inv[node], 2 + lt, ql] += b * np.float32(Z_DESCALE)
    return S


def _prep_weights(w_d, w_l, n_trees=N_TREES):
    fp8 = ml_dtypes.float8_e4m3
    w_l = np.asarray(w_l, dtype=np.float32)
    m = w_l.max(axis=-1, keepdims=True)
    e = np.exp(w_l - m, dtype=np.float32)
    sm = e / e.sum(axis=-1, keepdims=True)
    w2 = (sm[:, 0::2, :] + sm[:, 1::2, :]) * np.float32(1.0 / n_trees)
    # output-column error from fp8 leaf weights scales with the column's
    # magnitude: keep the top-NF16 columns (by L2 over trees+leaves) in f16,
    # the rest in fp8.  colperm = [f16 group | fp8 group]; the caller
    # un-permutes the gathered output.
    col_score = np.sqrt((w2.astype(np.float64) ** 2).sum(axis=(0, 1)))
    colperm = np.argsort(-col_score)
    w2 = w2[:, :, colperm]
    # [t, 256, C] -> pre-tiled [t, 128, 2, C]
    w2 = np.ascontiguousarray(
        w2.reshape(n_trees, 2, 128, CLASSES).transpose(0, 2, 1, 3)
    )
    w2h_np = w2[:, :, :, :NF16].astype(np.float16)
    w28_np = np.ascontiguousarray(
        w2[:, :, :, NF16:] * np.float32(W2_SCALE8)
    ).astype(fp8)
    wd_p = np.zeros((n_trees, IN_DIM, ND_PAD), np.float32)
    wd_p[:, :, : w_d.shape[2]] = w_d * np.float32(WD_SCALE)
    # permute decision-node columns so node-tile lt serves leaf-half lt
    perm = np.array(_node_perm()).reshape(-1)
    wd_p = wd_p[:, :, perm]
    # [t, (k p), d] -> pre-tiled [t, 128, k, d], split fp8/f16 k ranges
    wd_t = wd_p.reshape(n_trees, KI, 128, ND_PAD).transpose(0, 2, 1, 3)
    wd_8 = np.ascontiguousarray(wd_t[:, :, :K8]).astype(fp8)
    wd_16 = np.ascontiguousarray(wd_t[:, :, K8:]).astype(np.float16)
    return wd_8, wd_16, _smat_np(), w2h_np, w28_np, colperm


last_bass_results = None


def kernel(x, w_d, w_l):
    global last_bass_results
    x = np.asarray(x)
    wd_8, wd_16, S, w2h_np, w28_np, colperm = _prep_weights(
        np.asarray(w_d), np.asarray(w_l)
    )
    xs = x * np.float32(X_SCALE)
    in_maps = []
    for c in range(N_CORES):
        # [b_loc, IN_DIM] -> [128, n_chunks, k, CHUNK]:
        # xt[p, ci, k, n] = xs[c*B_LOC + ci*CHUNK + n, k*128 + p]
        xc = xs[c * B_LOC : (c + 1) * B_LOC, :]
        xct = xc.reshape(N_CHUNKS, CHUNK, KI, 128).transpose(3, 0, 2, 1)
        x_8 = np.ascontiguousarray(xct[:, :, :K8]).astype(ml_dtypes.float8_e4m3)
        x_16 = np.ascontiguousarray(xct[:, :, K8:]).astype(np.float16)
        in_maps.append(
            {"xt8": x_8, "xt16": x_16, "wd8": wd_8, "wd16": wd_16,
             "smat": S, "w2h": w2h_np, "w28": w28_np}
        )
    if "nc" not in _CACHE:
        _CACHE["nc"] = _build()
    res = run_bass_kernel_spmd(_CACHE["nc"], in_maps, core_ids=list(range(N_CORES)))
    last_bass_results = res
    perm_out = np.concatenate(
        [res.results[c]["out"] for c in range(N_CORES)], axis=0
    )
    out = np.empty_like(perm_out)
    out[:, colperm] = perm_out
    return out
